# revision 1
# baseline (speedup 1.0000x reference)
"""Trainium2 Bass kernel for the spectral-gating network (nn_DAPSO).

Model (B=4, C=256, H=W=256):
  - channels 0:128   : y_h = irfft(Gh * rfft(x, axis=H))   (per-channel gate)
  - channels 128:256 : y_w = irfft(Gw * rfft(x, axis=W))
  - gates Gh/Gw from tiny MLPs (computed on device)
  - channel attention: s = sigmoid(dw(gelu(W1 @ mean_hw(y) + b)))  -> y *= s
  - y2 = gelu(BN(lc_w @ y));  out = x + y2

Key algorithmic mapping: irfft(G*rfft(x)) along an axis of length N equals
T^T diag(ghat) T x with T the orthonormal real DFT basis (cos/sin rows), so
both branches become dense TensorE matmuls (no FFT).

Sharding: 8 cores = 4 batches x 2 w-halves. Each core computes BOTH branch
outputs for its (batch, w-half) spatial region: the H-branch needs only its
w-columns; the W-branch contracts the full W axis (its forward transform is
duplicated between the pair of cores). The only cross-core communication is
a 1KB AllReduce of the pooled channel means.

Channel attention pooling is computed analytically from input sums (DC
coefficient trick), so it never blocks on branch outputs:
  sum_h y_h = ghat_h[0] * sum_h x          (full-axis transform)
  sum_{w in Ws} y_w = sum_k sig[k] ghat_w[k, c] (T xsum_w)[k, c]

The branch outputs stream through a DRAM scratch in (c, h, w) layout; the
1x1 conv reads them back channel-major. The residual is applied by
pre-copying x into the output buffer and accumulating gelu(BN(conv)) with
CCE accumulate-DMAs.

Per-core layouts (host-prepped):
  xh   (256,128,128) bf16  [h, c, w]     HC-branch input slice
  xw   (256,128,256) bf16  [w, c, h]     WC-branch input (full w)
  xres (256,256,128) f32   [c, h, w]     residual slice (natural layout)
  out  (256,256,128) f32   [c, h, w]
"""
import sys
import os

sys.path.insert(0, "/opt/trn_rl_repo")

import numpy as np
import ml_dtypes

import concourse.bacc as bacc
import concourse.mybir as mybir
import concourse.tile as tile
from concourse import bass_utils

F32 = mybir.dt.float32
BF16 = mybir.dt.bfloat16
AF = mybir.ActivationFunctionType
ALU = mybir.AluOpType

N = 256          # H = W
C2 = 128         # channels per branch
B = 4
NCORES = 8
WS = 128         # per-core w-slice width

_BF16_NP = ml_dtypes.bfloat16


def _dft_basis():
    """Orthonormal real DFT basis T (N, N): y = T^T diag(ghat) T x == irfft(G*rfft(x))."""
    n = np.arange(N)
    k = np.arange(1, N // 2)
    T = np.zeros((N, N), np.float64)
    T[0, :] = 1.0 / np.sqrt(N)
    T[1:N // 2, :] = np.sqrt(2.0 / N) * np.cos(2 * np.pi * k[:, None] * n[None, :] / N)
    T[N // 2, :] = (1.0 / np.sqrt(N)) * ((-1.0) ** n)
    T[N // 2 + 1:, :] = np.sqrt(2.0 / N) * np.sin(2 * np.pi * k[:, None] * n[None, :] / N)
    return T.astype(np.float32)


def _part_major(a):
    """(256, ...) -> (128, 2, ...) partition-major layout."""
    a = np.asarray(a)
    return np.ascontiguousarray(a.reshape(2, 128, *a.shape[1:]).transpose(
        (1, 0) + tuple(range(2, a.ndim + 1))))


_MLPS = ("ah", "bc1", "aw", "bc2")


def _build():
    nc = bacc.Bacc("TRN2", target_bir_lowering=False, num_devices=NCORES)

    # ---------------- I/O declarations ----------------
    xh_d = nc.dram_tensor("xh", [256, 128, 128], BF16, kind="ExternalInput")
    xw_d = nc.dram_tensor("xw", [256, 128, 256], BF16, kind="ExternalInput")
    xres_d = nc.dram_tensor("xres", [256, 256, 128], F32, kind="ExternalInput")
    tfwd_d = nc.dram_tensor("tfwd", [128, 2, 256], BF16, kind="ExternalInput")
    tinv_d = nc.dram_tensor("tinv", [128, 2, 256], BF16, kind="ExternalInput")
    tinvw_d = nc.dram_tensor("tinvw", [128, 2, 128], BF16, kind="ExternalInput")
    sigw_d = nc.dram_tensor("sigw", [128, 2], F32, kind="ExternalInput")
    omega_d = nc.dram_tensor("omega", [1, 129], F32, kind="ExternalInput")
    lam_d = nc.dram_tensor("lam", [1, 128], F32, kind="ExternalInput")
    mlp_d = {}
    for m in _MLPS:
        mlp_d[m] = dict(
            w1t=nc.dram_tensor(f"{m}_w1t", [1, 64], F32, kind="ExternalInput"),
            b1=nc.dram_tensor(f"{m}_b1v", [64, 1], F32, kind="ExternalInput"),
            w2t=nc.dram_tensor(f"{m}_w2t", [64, 64], F32, kind="ExternalInput"),
            b2=nc.dram_tensor(f"{m}_b2v", [64, 1], F32, kind="ExternalInput"),
            w3t=nc.dram_tensor(f"{m}_w3t", [64, 8], F32, kind="ExternalInput"),
            b3=nc.dram_tensor(f"{m}_b3v", [8, 1], F32, kind="ExternalInput"),
        )
    caw1t_d = nc.dram_tensor("caw1t", [128, 2, 256], F32, kind="ExternalInput")
    cab1_d = nc.dram_tensor("cab1", [128, 2], F32, kind="ExternalInput")
    dwc_d = nc.dram_tensor("dwc", [128, 2], F32, kind="ExternalInput")
    dwb_d = nc.dram_tensor("dwb", [128, 2], F32, kind="ExternalInput")
    lcwt_d = nc.dram_tensor("lcwt", [128, 2, 256], F32, kind="ExternalInput")
    bng_d = nc.dram_tensor("bng", [128, 2], F32, kind="ExternalInput")
    bnb_d = nc.dram_tensor("bnb", [128, 2], F32, kind="ExternalInput")
    bnm_d = nc.dram_tensor("bnm", [128, 2], F32, kind="ExternalInput")
    bnv_d = nc.dram_tensor("bnv", [128, 2], F32, kind="ExternalInput")

    out_d = nc.dram_tensor("out", [256, 256, 128], F32, kind="ExternalOutput")

    ysc_d = nc.dram_tensor("ysc", [256, 256, 128], BF16)   # branch outputs (c, h, w)
    arh_in = nc.dram_tensor("arh_in", [128, 1], F32)
    arh_out = nc.dram_tensor("arh_out", [128, 1], F32)
    arw_in = nc.dram_tensor("arw_in", [128, 1], F32)
    arw_out = nc.dram_tensor("arw_out", [128, 1], F32)

    with tile.TileContext(nc) as tc:
        with tc.tile_pool(name="consts", bufs=1) as consts, \
             tc.tile_pool(name="xin", bufs=4) as xin, \
             tc.tile_pool(name="uch", bufs=3) as uch, \
             tc.tile_pool(name="stg", bufs=2) as stg, \
             tc.tile_pool(name="crhs", bufs=4) as crhs, \
             tc.tile_pool(name="outp", bufs=2) as outp, \
             tc.tile_pool(name="gsb", bufs=1) as gsb, \
             tc.tile_pool(name="ps", bufs=2, space="PSUM") as ps:

            # ---------------- const loads ----------------
            tfwd_t = consts.tile([128, 2, 256], BF16, tag="tfwd")
            nc.sync.dma_start(out=tfwd_t, in_=tfwd_d[:])
            tinv_t = consts.tile([128, 2, 256], BF16, tag="tinv")
            nc.sync.dma_start(out=tinv_t, in_=tinv_d[:])
            tinvw_t = consts.tile([128, 2, 128], BF16, tag="tinvw")
            nc.sync.dma_start(out=tinvw_t, in_=tinvw_d[:])
            sigw_t = consts.tile([128, 2], F32, tag="sigw")
            nc.sync.dma_start(out=sigw_t, in_=sigw_d[:])
            caw1t_t = consts.tile([128, 2, 256], F32, tag="caw1t")
            nc.sync.dma_start(out=caw1t_t, in_=caw1t_d[:])
            lcwt_t = consts.tile([128, 2, 256], F32, tag="lcwt")
            nc.sync.dma_start(out=lcwt_t, in_=lcwt_d[:])
            vec_t = {}
            for nm, d in (("cab1", cab1_d), ("dwc", dwc_d), ("dwb", dwb_d),
                          ("bng", bng_d), ("bnb", bnb_d), ("bnm", bnm_d), ("bnv", bnv_d)):
                vt = consts.tile([128, 2], F32, tag=f"v_{nm}")
                nc.sync.dma_start(out=vt, in_=d[:])
                vec_t[nm] = vt
            omega_t = consts.tile([1, 129], F32, tag="omega")
            nc.sync.dma_start(out=omega_t, in_=omega_d[:])
            lam_t = consts.tile([1, 128], F32, tag="lam")
            nc.sync.dma_start(out=lam_t, in_=lam_d[:])
            ones_t = consts.tile([128, 1], F32, tag="ones")
            nc.vector.memset(ones_t, 1.0)
            one1_t = consts.tile([1, 1], F32, tag="one1")
            nc.vector.memset(one1_t, 1.0)

            # residual pre-copy out <- x on the scalar HWDGE queue (keeps the
            # sync queue free for compute-feeding loads)
            for q in range(4):
                nc.scalar.dma_start(out=out_d[q * 64:(q + 1) * 64],
                                    in_=xres_d[q * 64:(q + 1) * 64])

            # ---------------- gate MLPs (tiny), table-batched ----------------
            def mlp_head(m, xvec, nk, role):
                d = mlp_d[m]
                w1t = gsb.tile([1, 64], F32, tag="m_w1")
                nc.sync.dma_start(out=w1t, in_=d["w1t"][:])
                b1 = gsb.tile([64, 1], F32, tag="m_b1")
                nc.sync.dma_start(out=b1, in_=d["b1"][:])
                w2t = gsb.tile([64, 64], F32, tag="m_w2")
                nc.sync.dma_start(out=w2t, in_=d["w2t"][:])
                b2 = gsb.tile([64, 1], F32, tag="m_b2")
                nc.sync.dma_start(out=b2, in_=d["b2"][:])
                w3t = gsb.tile([64, 8], F32, tag="m_w3")
                nc.sync.dma_start(out=w3t, in_=d["w3t"][:])
                b3 = gsb.tile([8, 1], F32, tag="m_b3")
                nc.sync.dma_start(out=b3, in_=d["b3"][:])

                p1 = ps.tile([64, nk], F32, tag="B0")
                nc.tensor.matmul(p1, lhsT=w1t, rhs=xvec, start=True, stop=True)
                h1 = gsb.tile([64, nk], F32, tag="m_h1")
                nc.scalar.activation(h1, p1, AF.Gelu, bias=b1)
                p2 = ps.tile([64, nk], F32, tag="B1")
                nc.tensor.matmul(p2, lhsT=w2t, rhs=h1, start=True, stop=True)
                h2 = gsb.tile([64, nk], F32, tag="m_h2")
                nc.scalar.activation(h2, p2, AF.Gelu, bias=b2)
                p3 = ps.tile([8, nk], F32, tag="B0")
                nc.tensor.matmul(p3, lhsT=w3t, rhs=h2, start=True, stop=True)
                at = gsb.tile([8, nk], F32, tag=f"m_at{role}")
                nc.scalar.activation(at, p3, AF.Identity, bias=b3)
                return at

            ghh = consts.tile([128, 2, 128], F32, tag="ghh")
            ghw = consts.tile([128, 2, 128], F32, tag="ghw")
            gp = {}
            gtags = {("h", 0): "A0", ("h", 1): "A1", ("w", 0): "B0", ("w", 1): "B1"}
            for (am, bm, nmk) in (("aw", "bc2", "w"), ("ah", "bc1", "h")):
                at = mlp_head(am, omega_t, 129, "a")
                bt = mlp_head(bm, lam_t, 128, "b")
                g0 = ps.tile([128, 128], F32, tag=gtags[(nmk, 0)])
                nc.tensor.matmul(g0, lhsT=at[:, 0:128], rhs=bt, start=True, stop=True)
                gn = ps.tile([1, 128], F32, tag=gtags[(nmk, 1)])
                nc.tensor.matmul(gn, lhsT=at[:, 128:129], rhs=bt, start=True, stop=True)
                gp[(nmk, 0)] = g0
                gp[(nmk, 1)] = gn
            # softplus(z) = relu(z) + log1p(exp(-|z|)), stage-batched across all 4
            keys = list(gp.keys())
            sp = {}
            for i, key in enumerate(keys):
                npart = 128 if key[1] == 0 else 1
                na = gsb.tile([128, 128], F32, tag=f"sp_na{i}")
                nc.scalar.activation(na[:npart, :], gp[key], AF.Abs)
                sp[key] = na
            for i, key in enumerate(keys):
                npart = 128 if key[1] == 0 else 1
                ex = gsb.tile([128, 128], F32, tag=f"sp_ex{i}")
                nc.scalar.activation(ex[:npart, :], sp[key][:npart, :], AF.Exp, scale=-1.0)
                nc.vector.tensor_scalar_add(ex[:npart, :], ex[:npart, :], 1.0)
                sp[key] = ex
            for key in keys:
                npart = 128 if key[1] == 0 else 1
                nc.scalar.activation(sp[key][:npart, :], sp[key][:npart, :], AF.Ln)
            for i, key in enumerate(keys):
                npart = 128 if key[1] == 0 else 1
                re = gsb.tile([128, 128], F32, tag=f"sp_re{i}")
                nc.scalar.activation(re[:npart, :], gp[key], AF.Relu)
                gh = ghh if key[0] == "h" else ghw
                if key[1] == 0:
                    nc.vector.tensor_add(gh[:, 0, :], sp[key][:128, :], re[:128, :])
                else:
                    # rows 128+j of ghat equal G[j]: copy the aligned block first,
                    # then overwrite row 0 with the Nyquist G[128].
                    nc.vector.tensor_copy(gh[:, 1, :], gh[:, 0, :])
                    nc.vector.tensor_add(gh[0:1, 1, :], sp[key][0:1, :], re[0:1, :])
            for gh in (ghh, ghw):
                nc.vector.tensor_scalar_mul(gh[:, :, :], gh[:, :, :], float(8.0 ** -0.5))

            # ---------------- BN prep ----------------
            bninv = consts.tile([128, 2], F32, tag="bninv")
            nc.vector.tensor_scalar_add(bninv, vec_t["bnv"], 1e-5)
            nc.scalar.activation(bninv, bninv, AF.Sqrt)
            nc.vector.reciprocal(bninv, bninv)
            nc.vector.tensor_tensor(out=bninv, in0=vec_t["bng"], in1=bninv, op=ALU.mult)
            bnbeff = consts.tile([128, 2], F32, tag="bnbeff")
            nc.vector.tensor_tensor(out=bnbeff, in0=vec_t["bnm"], in1=bninv, op=ALU.mult)
            nc.vector.tensor_tensor(out=bnbeff, in0=vec_t["bnb"], in1=bnbeff, op=ALU.subtract)

            xsum_h = consts.tile([128, 2, 128], F32, tag="xsumh")      # [h, ht, c]
            xsum_w = consts.tile([128, 2, 128], F32, tag="xsumw")      # [w, wt, c]

            # ---------------- WC branch (first: its pooled sums gate the AR) ----
            for cs in range(0, 128, 8):
                xw_t = []
                for wt in (0, 1):
                    xt = xin.tile([128, 8, 256], BF16, tag=f"xb{wt}")
                    nc.sync.dma_start(out=xt, in_=xw_d[wt * 128:(wt + 1) * 128, cs:cs + 8, :])
                    xw_t.append(xt)
                    # h-sum via GPSIMD tree-adds (keeps DVE free)
                    tr1 = gsb.tile([128, 8, 128], BF16, tag="tree1")
                    nc.gpsimd.tensor_add(tr1, xt[:, :, 0:128], xt[:, :, 128:256])
                    tr2 = gsb.tile([128, 8, 64], BF16, tag="tree2")
                    nc.gpsimd.tensor_add(tr2, tr1[:, :, 0:64], tr1[:, :, 64:128])
                    tr3 = gsb.tile([128, 8, 32], BF16, tag="tree3")
                    nc.gpsimd.tensor_add(tr3, tr2[:, :, 0:32], tr2[:, :, 32:64])
                    tr4 = gsb.tile([128, 8, 16], BF16, tag="tree4")
                    nc.gpsimd.tensor_add(tr4, tr3[:, :, 0:16], tr3[:, :, 16:32])
                    nc.vector.tensor_reduce(out=xsum_w[:, wt, cs:cs + 8], in_=tr4,
                                            axis=mybir.AxisListType.X, op=ALU.add)
                ystg = []
                for ht in (0, 1):
                    st = stg.tile([128, 8, 128], BF16, tag=f"ystgw{ht}")
                    ystg.append(st)
                for cc in range(0, 8, 2):
                    c0 = cs + cc
                    puw = []
                    for kt in (0, 1):
                        pk = ps.tile([128, 2, 256], F32, tag=f"A{kt}")
                        for wt in (0, 1):
                            nc.tensor.matmul(pk, lhsT=tfwd_t[:, wt, kt * 128:(kt + 1) * 128],
                                             rhs=xw_t[wt][:, cc:cc + 2, :],
                                             start=(wt == 0), stop=(wt == 1))
                        puw.append(pk)
                    ugw = []
                    for kt in (0, 1):
                        u = uch.tile([128, 2, 256], BF16, tag=f"uw{kt}")
                        nc.vector.tensor_tensor(
                            out=u, in0=puw[kt],
                            in1=ghw[:, kt, c0:c0 + 2].unsqueeze(2).broadcast_to([128, 2, 256]),
                            op=ALU.mult)
                        ugw.append(u)
                    for ht in (0, 1):
                        pyw = ps.tile([128, 2, 128], F32, tag=f"B{ht}")
                        for c2 in (0, 1):
                            for kt in (0, 1):
                                nc.tensor.matmul(pyw[:, c2, :],
                                                 lhsT=ugw[kt][:, c2, ht * 128:(ht + 1) * 128],
                                                 rhs=tinvw_t[:, kt, :],
                                                 start=(kt == 0), stop=(kt == 1))
                        if ht == 0:
                            nc.vector.tensor_copy(ystg[ht][:, cc:cc + 2, :], pyw)
                        else:
                            nc.scalar.activation(ystg[ht][:, cc:cc + 2, :], pyw, AF.Copy)
                for ht in (0, 1):
                    nc.sync.dma_start(
                        out=ysc_d[128 + cs:128 + cs + 8, ht * 128:(ht + 1) * 128, :]
                        .rearrange("c h w -> h c w"),
                        in_=ystg[ht])

            # pool_w = sum_k sigw[k] ghw[k, c] * (T @ xsum_w)[k, c]  -> AllReduce #1
            xsum_wb = gsb.tile([128, 2, 128], BF16, tag="xsumwb")
            nc.vector.tensor_copy(xsum_wb, xsum_w)
            t1 = []
            for kt in (0, 1):
                m1 = ps.tile([128, 128], F32, tag=f"B{kt}")
                for wt in (0, 1):
                    nc.tensor.matmul(m1, lhsT=tfwd_t[:, wt, kt * 128:(kt + 1) * 128],
                                     rhs=xsum_wb[:, wt, :], start=(wt == 0), stop=(wt == 1))
                tt = gsb.tile([128, 128], F32, tag=f"t1_{kt}")
                nc.vector.tensor_tensor(out=tt, in0=m1, in1=ghw[:, kt, :], op=ALU.mult)
                t1.append(tt)
            pw_ps = ps.tile([128, 1], F32, tag="A0")
            for kt in (0, 1):
                nc.tensor.matmul(pw_ps, lhsT=t1[kt], rhs=sigw_t[:, kt:kt + 1],
                                 start=(kt == 0), stop=(kt == 1))
            poolw_sb = gsb.tile([128, 1], F32, tag="poolw")
            nc.vector.tensor_copy(poolw_sb, pw_ps)
            nc.sync.dma_start(out=arw_in[:], in_=poolw_sb)
            nc.gpsimd.collective_compute(
                "AllReduce", ALU.add,
                replica_groups=[[0, 1], [2, 3], [4, 5], [6, 7]],
                ins=[arw_in[:]], outs=[arw_out[:]])

            # ---------------- HC branch ----------------
            for cs in range(0, 128, 8):
                xh_t = []
                for ht in (0, 1):
                    xt = xin.tile([128, 8, 128], BF16, tag=f"xa{ht}")
                    nc.sync.dma_start(out=xt, in_=xh_d[ht * 128:(ht + 1) * 128, cs:cs + 8, :])
                    xh_t.append(xt)
                    nc.vector.tensor_reduce(out=xsum_h[:, ht, cs:cs + 8], in_=xt,
                                            axis=mybir.AxisListType.X, op=ALU.add)
                ystg = []
                for ht in (0, 1):
                    st = stg.tile([128, 8, 128], BF16, tag=f"ystgh{ht}")
                    ystg.append(st)
                for cc in range(0, 8, 4):
                    gc = cs + cc
                    pu = []
                    for kt in (0, 1):
                        pk = ps.tile([128, 4, 128], F32, tag=f"A{kt}")
                        for ht in (0, 1):
                            nc.tensor.matmul(pk, lhsT=tfwd_t[:, ht, kt * 128:(kt + 1) * 128],
                                             rhs=xh_t[ht][:, cc:cc + 4, :],
                                             start=(ht == 0), stop=(ht == 1))
                        pu.append(pk)
                    ug = []
                    for kt in (0, 1):
                        u = uch.tile([128, 4, 128], BF16, tag=f"ug{kt}")
                        nc.vector.tensor_tensor(
                            out=u, in0=pu[kt],
                            in1=ghh[:, kt, gc:gc + 4].unsqueeze(2).broadcast_to([128, 4, 128]),
                            op=ALU.mult)
                        ug.append(u)
                    for ht in (0, 1):
                        py = ps.tile([128, 4, 128], F32, tag=f"B{ht}")
                        for kt in (0, 1):
                            nc.tensor.matmul(py, lhsT=tinv_t[:, kt, ht * 128:(ht + 1) * 128],
                                             rhs=ug[kt], start=(kt == 0), stop=(kt == 1))
                        if ht == 0:
                            nc.vector.tensor_copy(ystg[ht][:, cc:cc + 4, :], py)
                        else:
                            nc.scalar.activation(ystg[ht][:, cc:cc + 4, :], py, AF.Copy)
                for ht in (0, 1):
                    nc.sync.dma_start(
                        out=ysc_d[cs:cs + 8, ht * 128:(ht + 1) * 128, :]
                        .rearrange("c h w -> h c w"),
                        in_=ystg[ht])

            # pool_h = ghh[0, :] * sum_{h,w} xh  -> AllReduce #2
            ph_ps = ps.tile([128, 1], F32, tag="A0")
            for ht in (0, 1):
                nc.tensor.matmul(ph_ps, lhsT=xsum_h[:, ht, :], rhs=ones_t,
                                 start=(ht == 0), stop=(ht == 1))
            g0_ps = ps.tile([128, 1], F32, tag="A1")
            nc.tensor.matmul(g0_ps, lhsT=ghh[0:1, 0, :], rhs=one1_t, start=True, stop=True)
            g0_sb = gsb.tile([128, 1], F32, tag="g0sb")
            nc.vector.tensor_copy(g0_sb, g0_ps)
            poolh_sb = gsb.tile([128, 1], F32, tag="poolh")
            nc.vector.tensor_tensor(out=poolh_sb, in0=ph_ps, in1=g0_sb, op=ALU.mult)
            nc.sync.dma_start(out=arh_in[:], in_=poolh_sb)
            nc.gpsimd.collective_compute(
                "AllReduce", ALU.add,
                replica_groups=[[0, 1], [2, 3], [4, 5], [6, 7]],
                ins=[arh_in[:]], outs=[arh_out[:]])

            p_sb = []
            for ct, aro in ((0, arh_out), (1, arw_out)):
                pt = gsb.tile([128, 1], F32, tag=f"p_ar{ct}")
                nc.sync.dma_start(out=pt, in_=aro[:])
                p_sb.append(pt)

            # ---------------- channel attention -> folded conv weights ----------------
            q_sb = []
            for ot in (0, 1):
                q_ps = ps.tile([128, 1], F32, tag=f"B{ot}")
                for ct in (0, 1):
                    nc.tensor.matmul(q_ps, lhsT=caw1t_t[:, ct, ot * 128:(ot + 1) * 128],
                                     rhs=p_sb[ct], start=(ct == 0), stop=(ct == 1))
                qt = gsb.tile([128, 1], F32, tag=f"q{ot}")
                nc.scalar.activation(qt, q_ps, AF.Gelu, bias=vec_t["cab1"][:, ot:ot + 1])
                nc.vector.tensor_tensor(out=qt, in0=qt, in1=vec_t["dwc"][:, ot:ot + 1],
                                        op=ALU.mult)
                q_sb.append(qt)
            s_sb = []
            for ot in (0, 1):
                s_t = gsb.tile([128, 1], F32, tag=f"s{ot}")
                nc.scalar.activation(s_t, q_sb[ot], AF.Sigmoid, bias=vec_t["dwb"][:, ot:ot + 1])
                s_sb.append(s_t)
            wsc = consts.tile([128, 2, 256], BF16, tag="wsc")
            for ct in (0, 1):
                nc.vector.tensor_scalar_mul(wsc[:, ct, :], lcwt_t[:, ct, :], s_sb[ct])

            # ---------------- conv 1x1 + BN + GELU + residual-accum ----------------
            HCH = 8
            for h0 in range(0, 256, HCH):
                rts = []
                for ct in (0, 1):
                    rt = crhs.tile([128, HCH, 128], BF16, tag=f"cr{ct}")
                    nc.sync.dma_start(out=rt, in_=ysc_d[ct * 128:(ct + 1) * 128,
                                                        h0:h0 + HCH, :])
                    rts.append(rt)
                for ot in (0, 1):
                    gstg = outp.tile([128, HCH, 128], F32, tag=f"gstg{ot}")
                    for sl in range(0, HCH, 4):
                        po = ps.tile([128, 4, 128], F32, tag=f"A{ot}")
                        for ct in (0, 1):
                            nc.tensor.matmul(po, lhsT=wsc[:, ct, ot * 128:(ot + 1) * 128],
                                             rhs=rts[ct][:, sl:sl + 4, :],
                                             start=(ct == 0), stop=(ct == 1))
                        nc.scalar.activation(gstg[:, sl:sl + 4, :], po, AF.Gelu,
                                             bias=bnbeff[:, ot:ot + 1],
                                             scale=bninv[:, ot:ot + 1])
                    nc.gpsimd.dma_start(out=out_d[ot * 128:(ot + 1) * 128, h0:h0 + HCH, :],
                                        in_=gstg, accum_op=ALU.add)

    nc.compile()
    return nc


_NC_CACHE = None


def _get_nc():
    global _NC_CACHE
    if _NC_CACHE is None:
        _NC_CACHE = _build()
    return _NC_CACHE


def _host_consts(inputs, core):
    """Per-core constant inputs (everything except the x shards)."""
    s = core % 2
    wlo = WS * s
    T = _dft_basis()
    d = {}
    d["tfwd"] = _part_major(np.ascontiguousarray(T.T)).astype(_BF16_NP)
    d["tinv"] = _part_major(T).astype(_BF16_NP)
    d["tinvw"] = _part_major(np.ascontiguousarray(T[:, wlo:wlo + WS])).astype(_BF16_NP)
    d["sigw"] = _part_major(T[:, wlo:wlo + WS].sum(axis=1)).astype(np.float32)
    d["omega"] = (np.arange(129, dtype=np.float32) / 128.0 - 1.0).reshape(1, 129)
    d["lam"] = np.linspace(-1.0, 1.0, 128, dtype=np.float32).reshape(1, 128)
    for m in _MLPS:
        d[f"{m}_w1t"] = np.ascontiguousarray(inputs[f"{m}_w1"].T).astype(np.float32)
        d[f"{m}_b1v"] = inputs[f"{m}_b1"].reshape(64, 1).astype(np.float32)
        d[f"{m}_w2t"] = np.ascontiguousarray(inputs[f"{m}_w2"].T).astype(np.float32)
        d[f"{m}_b2v"] = inputs[f"{m}_b2"].reshape(64, 1).astype(np.float32)
        d[f"{m}_w3t"] = np.ascontiguousarray(inputs[f"{m}_w3"].T).astype(np.float32)
        d[f"{m}_b3v"] = inputs[f"{m}_b3"].reshape(8, 1).astype(np.float32)
    d["caw1t"] = _part_major(np.ascontiguousarray(inputs["ca_w1"].T) / 65536.0).astype(np.float32)
    d["cab1"] = _part_major(inputs["ca_b1"]).astype(np.float32)
    d["dwc"] = _part_major(np.ascontiguousarray(inputs["ca_dw"][:, 1, 1])).astype(np.float32)
    d["dwb"] = _part_major(inputs["ca_db"]).astype(np.float32)
    d["lcwt"] = _part_major(np.ascontiguousarray(inputs["lc_w"].T)).astype(np.float32)
    d["bng"] = _part_major(inputs["bn_g"]).astype(np.float32)
    d["bnb"] = _part_major(inputs["bn_b"]).astype(np.float32)
    d["bnm"] = _part_major(inputs["bn_m"]).astype(np.float32)
    d["bnv"] = _part_major(inputs["bn_v"]).astype(np.float32)
    return d


def kernel(**inputs):
    x = np.asarray(inputs["x"], np.float32)
    nc = _get_nc()

    in_maps = []
    for core in range(NCORES):
        b, s = core // 2, core % 2
        wlo = WS * s
        m = _host_consts(inputs, core)
        m["xh"] = np.ascontiguousarray(
            x[b, :C2, :, wlo:wlo + WS].transpose(1, 0, 2)).astype(_BF16_NP)
        m["xw"] = np.ascontiguousarray(
            x[b, C2:, :, :].transpose(2, 0, 1)).astype(_BF16_NP)
        m["xres"] = np.ascontiguousarray(x[b, :, :, wlo:wlo + WS])
        in_maps.append(m)

    trace = os.environ.get("BASS_KERNEL_TRACE", "0") == "1"
    res = bass_utils.run_bass_kernel_spmd(
        nc, in_maps, core_ids=list(range(NCORES)),
        trace=trace, trace_cores=list(range(NCORES)) if trace else None,
        stitch_traces=False)
    if trace and res.exec_time_ns is not None:
        print(f"HW exec time: {res.exec_time_ns} ns")
        print(f"   mean exec time: {res.mean_exec_time_ns} ns  "
              f"(slowest core {res.max_exec_time_core_id})")
        if res.instructions_and_trace is not None:
            print("   trace:", res.instructions_and_trace[1])

    out = np.empty((B, 2 * C2, N, N), np.float32)
    for core in range(NCORES):
        b, s = core // 2, core % 2
        wlo = WS * s
        out[b, :, :, wlo:wlo + WS] = res.results[core]["out"]
    return out



# revision 13
# speedup vs baseline: 1.0008x; 1.0008x over previous
"""Trainium2 Bass kernel for the spectral-gating network (nn_DAPSO).

Model (B=4, C=256, H=W=256):
  - channels 0:128   : y_h = irfft(Gh * rfft(x, axis=H))   (per-channel gate)
  - channels 128:256 : y_w = irfft(Gw * rfft(x, axis=W))
  - gates Gh/Gw from tiny MLPs (computed on device)
  - channel attention: s = sigmoid(dw(gelu(W1 @ mean_hw(y) + b)))  -> y *= s
  - y2 = gelu(BN(lc_w @ y));  out = x + y2

irfft(G*rfft(x)) along an axis of length N equals T^T diag(ghat) T x with T
the orthonormal real DFT basis, so both branches are dense TensorE matmuls.

Sharding: 8 cores = 4 batches x 2 w-halves; only cross-core traffic is a
1KB AllReduce of pooled channel means (computed analytically from input
sums via the DC-coefficient trick, so it never blocks on branch outputs).

v2 (all-SBUF dataflow): the inverse transforms are data-stationary matmuls
(lhsT = gated-spectrum column, rhs = inverse basis row-block), which lands
the branch outputs channel-major in SBUF directly -- no DRAM scratch, no
layout-transposing DMA. The residual is added on-chip and the output is
written once, in bf16.  Per-core HBM traffic drops from ~160MB to ~73MB.

Phases (order chosen so the AllReduces overlap compute):
  gates -> [xw pre-sum pass on scalar DGE] -> HC fwd -> pool_w/AR-w ->
  pool_h/AR-h -> HC inv -> attention/conv-weight fold ->
  4x { WC fwd (h-quarter) -> WC inv -> conv+BN+GELU+residual -> store }
"""
import sys
import os

sys.path.insert(0, "/opt/trn_rl_repo")

import numpy as np
import ml_dtypes

import concourse.bacc as bacc
import concourse.mybir as mybir
import concourse.tile as tile
from concourse import bass_utils

F32 = mybir.dt.float32
BF16 = mybir.dt.bfloat16
AF = mybir.ActivationFunctionType
ALU = mybir.AluOpType
AXX = mybir.AxisListType.X

N = 256          # H = W
C2 = 128         # channels per branch
B = 4
NCORES = 8
WS = 128         # per-core w-slice width
HQ = 64          # WC h-quarter size

_BF16_NP = ml_dtypes.bfloat16


def _dft_basis():
    """Orthonormal real DFT basis T (N, N): y = T^T diag(ghat) T x == irfft(G*rfft(x))."""
    n = np.arange(N)
    k = np.arange(1, N // 2)
    T = np.zeros((N, N), np.float64)
    T[0, :] = 1.0 / np.sqrt(N)
    T[1:N // 2, :] = np.sqrt(2.0 / N) * np.cos(2 * np.pi * k[:, None] * n[None, :] / N)
    T[N // 2, :] = (1.0 / np.sqrt(N)) * ((-1.0) ** n)
    T[N // 2 + 1:, :] = np.sqrt(2.0 / N) * np.sin(2 * np.pi * k[:, None] * n[None, :] / N)
    return T.astype(np.float32)


def _part_major(a):
    """(256, ...) -> (128, 2, ...) partition-major layout."""
    a = np.asarray(a)
    return np.ascontiguousarray(a.reshape(2, 128, *a.shape[1:]).transpose(
        (1, 0) + tuple(range(2, a.ndim + 1))))


_MLPS = ("ah", "bc1", "aw", "bc2")


def _build():
    nc = bacc.Bacc("TRN2", target_bir_lowering=False, num_devices=NCORES)

    # ---------------- I/O declarations ----------------
    xh_d = nc.dram_tensor("xh", [256, 128, 128], BF16, kind="ExternalInput")
    xw_d = nc.dram_tensor("xw", [4, 2, 128, 128, HQ], BF16, kind="ExternalInput")
    xres_d = nc.dram_tensor("xres", [256, 256, 128], BF16, kind="ExternalInput")
    tfwd_d = nc.dram_tensor("tfwd", [128, 2, 256], BF16, kind="ExternalInput")
    tinv_d = nc.dram_tensor("tinv", [128, 2, 256], BF16, kind="ExternalInput")
    tinvw_d = nc.dram_tensor("tinvw", [128, 2, 128], BF16, kind="ExternalInput")
    sigw_d = nc.dram_tensor("sigw", [128, 2], F32, kind="ExternalInput")
    omega_d = nc.dram_tensor("omega", [1, 129], F32, kind="ExternalInput")
    lam_d = nc.dram_tensor("lam", [1, 128], F32, kind="ExternalInput")
    mlp_d = {}
    for m in _MLPS:
        mlp_d[m] = dict(
            w1t=nc.dram_tensor(f"{m}_w1t", [1, 64], F32, kind="ExternalInput"),
            b1=nc.dram_tensor(f"{m}_b1v", [64, 1], F32, kind="ExternalInput"),
            w2t=nc.dram_tensor(f"{m}_w2t", [64, 64], F32, kind="ExternalInput"),
            b2=nc.dram_tensor(f"{m}_b2v", [64, 1], F32, kind="ExternalInput"),
            w3t=nc.dram_tensor(f"{m}_w3t", [64, 8], F32, kind="ExternalInput"),
            b3=nc.dram_tensor(f"{m}_b3v", [8, 1], F32, kind="ExternalInput"),
        )
    caw1t_d = nc.dram_tensor("caw1t", [128, 2, 256], F32, kind="ExternalInput")
    cab1_d = nc.dram_tensor("cab1", [128, 2], F32, kind="ExternalInput")
    dwc_d = nc.dram_tensor("dwc", [128, 2], F32, kind="ExternalInput")
    dwb_d = nc.dram_tensor("dwb", [128, 2], F32, kind="ExternalInput")
    lcwt_d = nc.dram_tensor("lcwt", [128, 2, 256], BF16, kind="ExternalInput")
    bng_d = nc.dram_tensor("bng", [128, 2], F32, kind="ExternalInput")
    bnb_d = nc.dram_tensor("bnb", [128, 2], F32, kind="ExternalInput")
    bnm_d = nc.dram_tensor("bnm", [128, 2], F32, kind="ExternalInput")
    bnv_d = nc.dram_tensor("bnv", [128, 2], F32, kind="ExternalInput")

    out_d = nc.dram_tensor("out", [256, 256, 128], BF16, kind="ExternalOutput")

    arh_in = nc.dram_tensor("arh_in", [128, 1], F32)
    arh_out = nc.dram_tensor("arh_out", [128, 1], F32)
    arw_in = nc.dram_tensor("arw_in", [128, 1], F32)
    arw_out = nc.dram_tensor("arw_out", [128, 1], F32)

    with tile.TileContext(nc) as tc:
        with tc.tile_pool(name="consts", bufs=1) as consts, \
             tc.tile_pool(name="xin", bufs=2) as xin, \
             tc.tile_pool(name="xe", bufs=2) as xe, \
             tc.tile_pool(name="ubuf", bufs=1) as ubuf, \
             tc.tile_pool(name="ybuf", bufs=1) as ybuf, \
             tc.tile_pool(name="stg", bufs=2) as stg, \
             tc.tile_pool(name="gsb", bufs=1) as gsb, \
             tc.tile_pool(name="ps", bufs=2, space="PSUM") as ps:

            # ---------------- const loads ----------------
            tfwd_t = consts.tile([128, 2, 256], BF16, tag="tfwd")
            nc.sync.dma_start(out=tfwd_t, in_=tfwd_d[:])
            tinv_t = consts.tile([128, 2, 256], BF16, tag="tinv")
            nc.sync.dma_start(out=tinv_t, in_=tinv_d[:])
            tinvw_t = consts.tile([128, 2, 128], BF16, tag="tinvw")
            nc.sync.dma_start(out=tinvw_t, in_=tinvw_d[:])
            sigw_t = consts.tile([128, 2], F32, tag="sigw")
            nc.sync.dma_start(out=sigw_t, in_=sigw_d[:])
            caw1t_t = consts.tile([128, 2, 256], F32, tag="caw1t")
            nc.sync.dma_start(out=caw1t_t, in_=caw1t_d[:])
            lcwt_t = consts.tile([128, 2, 256], BF16, tag="lcwt")
            nc.sync.dma_start(out=lcwt_t, in_=lcwt_d[:])
            vec_t = {}
            for nm, d in (("cab1", cab1_d), ("dwc", dwc_d), ("dwb", dwb_d),
                          ("bng", bng_d), ("bnb", bnb_d), ("bnm", bnm_d), ("bnv", bnv_d)):
                vt = consts.tile([128, 2], F32, tag=f"v_{nm}")
                nc.sync.dma_start(out=vt, in_=d[:])
                vec_t[nm] = vt
            omega_t = consts.tile([1, 129], F32, tag="omega")
            nc.sync.dma_start(out=omega_t, in_=omega_d[:])
            lam_t = consts.tile([1, 128], F32, tag="lam")
            nc.sync.dma_start(out=lam_t, in_=lam_d[:])
            ones_t = consts.tile([128, 1], F32, tag="ones")
            nc.vector.memset(ones_t, 1.0)
            one1_t = consts.tile([1, 1], F32, tag="one1")
            nc.vector.memset(one1_t, 1.0)

            # ---------------- gate MLPs (tiny), table-batched ----------------
            def mlp_head(m, xvec, nk, role):
                d = mlp_d[m]
                w1t = gsb.tile([1, 64], F32, tag="m_w1")
                nc.sync.dma_start(out=w1t, in_=d["w1t"][:])
                b1 = gsb.tile([64, 1], F32, tag="m_b1")
                nc.sync.dma_start(out=b1, in_=d["b1"][:])
                w2t = gsb.tile([64, 64], F32, tag="m_w2")
                nc.sync.dma_start(out=w2t, in_=d["w2t"][:])
                b2 = gsb.tile([64, 1], F32, tag="m_b2")
                nc.sync.dma_start(out=b2, in_=d["b2"][:])
                w3t = gsb.tile([64, 8], F32, tag="m_w3")
                nc.sync.dma_start(out=w3t, in_=d["w3t"][:])
                b3 = gsb.tile([8, 1], F32, tag="m_b3")
                nc.sync.dma_start(out=b3, in_=d["b3"][:])

                p1 = ps.tile([64, nk], F32, tag="B0")
                nc.tensor.matmul(p1, lhsT=w1t, rhs=xvec, start=True, stop=True)
                h1 = gsb.tile([64, nk], F32, tag="m_h1")
                nc.scalar.activation(h1, p1, AF.Gelu, bias=b1)
                p2 = ps.tile([64, nk], F32, tag="B1")
                nc.tensor.matmul(p2, lhsT=w2t, rhs=h1, start=True, stop=True)
                h2 = gsb.tile([64, nk], F32, tag="m_h2")
                nc.scalar.activation(h2, p2, AF.Gelu, bias=b2)
                p3 = ps.tile([8, nk], F32, tag="B0")
                nc.tensor.matmul(p3, lhsT=w3t, rhs=h2, start=True, stop=True)
                at = gsb.tile([8, nk], F32, tag=f"m_at{role}")
                nc.scalar.activation(at, p3, AF.Identity, bias=b3)
                return at

            ghh = consts.tile([128, 2, 128], F32, tag="ghh")
            ghw = consts.tile([128, 2, 128], F32, tag="ghw")
            gp = {}
            gtags = {("h", 0): "A0", ("h", 1): "A1", ("w", 0): "B0", ("w", 1): "B1"}
            for (am, bm, nmk) in (("aw", "bc2", "w"), ("ah", "bc1", "h")):
                at = mlp_head(am, omega_t, 129, "a")
                bt = mlp_head(bm, lam_t, 128, "b")
                g0 = ps.tile([128, 128], F32, tag=gtags[(nmk, 0)])
                nc.tensor.matmul(g0, lhsT=at[:, 0:128], rhs=bt, start=True, stop=True)
                gn = ps.tile([1, 128], F32, tag=gtags[(nmk, 1)])
                nc.tensor.matmul(gn, lhsT=at[:, 128:129], rhs=bt, start=True, stop=True)
                gp[(nmk, 0)] = g0
                gp[(nmk, 1)] = gn
            # softplus(z) = relu(z) + log1p(exp(-|z|)), stage-batched across all 4
            keys = list(gp.keys())
            sp = {}
            for i, key in enumerate(keys):
                npart = 128 if key[1] == 0 else 1
                na = gsb.tile([128, 128], F32, tag=f"sp_na{i}")
                nc.scalar.activation(na[:npart, :], gp[key], AF.Abs)
                sp[key] = na
            for i, key in enumerate(keys):
                npart = 128 if key[1] == 0 else 1
                ex = gsb.tile([128, 128], F32, tag=f"sp_ex{i}")
                nc.scalar.activation(ex[:npart, :], sp[key][:npart, :], AF.Exp, scale=-1.0)
                nc.vector.tensor_scalar_add(ex[:npart, :], ex[:npart, :], 1.0)
                sp[key] = ex
            for key in keys:
                npart = 128 if key[1] == 0 else 1
                nc.scalar.activation(sp[key][:npart, :], sp[key][:npart, :], AF.Ln)
            for i, key in enumerate(keys):
                npart = 128 if key[1] == 0 else 1
                re = gsb.tile([128, 128], F32, tag=f"sp_na{i}")
                nc.scalar.activation(re[:npart, :], gp[key], AF.Relu)
                gh = ghh if key[0] == "h" else ghw
                if key[1] == 0:
                    nc.vector.tensor_add(gh[:, 0, :], sp[key][:128, :], re[:128, :])
                else:
                    # rows 128+j of ghat equal G[j]: copy the aligned block first,
                    # then overwrite row 0 with the Nyquist G[128].
                    nc.vector.tensor_copy(gh[:, 1, :], gh[:, 0, :])
                    nc.vector.tensor_add(gh[0:1, 1, :], sp[key][0:1, :], re[0:1, :])
            for gh in (ghh, ghw):
                nc.vector.tensor_scalar_mul(gh[:, :, :], gh[:, :, :], float(8.0 ** -0.5))

            # ---------------- BN prep ----------------
            bninv = consts.tile([128, 2], F32, tag="bninv")
            nc.vector.tensor_scalar_add(bninv, vec_t["bnv"], 1e-5)
            nc.scalar.activation(bninv, bninv, AF.Sqrt)
            nc.vector.reciprocal(bninv, bninv)
            nc.vector.tensor_tensor(out=bninv, in0=vec_t["bng"], in1=bninv, op=ALU.mult)
            bnbeff = consts.tile([128, 2], F32, tag="bnbeff")
            nc.vector.tensor_tensor(out=bnbeff, in0=vec_t["bnm"], in1=bninv, op=ALU.mult)
            nc.vector.tensor_tensor(out=bnbeff, in0=vec_t["bnb"], in1=bnbeff, op=ALU.subtract)

            xsum_h = consts.tile([128, 2, 128], F32, tag="xsumh")      # [h, ht, c]
            xsum_w = consts.tile([128, 2, 128], F32, tag="xsumw")      # [w, wt, c]

            # ---------------- early xw sum pass (scalar DGE queue) -----------
            # Streams all of xw once, tree-summing over h, so AR-w can fire
            # long before the WC branch compute needs its result.  Everything
            # runs on gpsimd (idle this early): any op on the in-order vector
            # or scalar queues would block later work behind this slow 16MB
            # stream.
            for q in range(4):
                for wt in (0, 1):
                    for cs in range(0, 128, 8):
                        xt = xe.tile([128, 8, HQ], BF16, tag=f"e{wt}")
                        nc.scalar.dma_start(out=xt, in_=xw_d[q, wt, :, cs:cs + 8, :])
                        tr = xt
                        for lvl in (32, 16, 8, 4, 2):
                            nt = gsb.tile([128, 8, lvl], BF16, tag=f"tr{lvl}")
                            nc.gpsimd.tensor_add(nt, tr[:, :, 0:lvl], tr[:, :, lvl:2 * lvl])
                            tr = nt
                        if q == 0:
                            nc.gpsimd.tensor_add(xsum_w[:, wt, cs:cs + 8],
                                                 tr[:, :, 0], tr[:, :, 1])
                        else:
                            rs = gsb.tile([128, 8], F32, tag="rsum")
                            nc.gpsimd.tensor_add(rs, tr[:, :, 0], tr[:, :, 1])
                            nc.gpsimd.tensor_add(xsum_w[:, wt, cs:cs + 8],
                                                 xsum_w[:, wt, cs:cs + 8], rs)

            # ---------------- HC forward: u = ghat * (T x) ----------------
            # u split in two w-half tiles (reused as WC h-quarter ping-pong)
            u0 = ubuf.tile([128, 2, 128, HQ], BF16, tag="u0")   # [k, kt, c, w0:64]
            u1 = ubuf.tile([128, 2, 128, HQ], BF16, tag="u1")   # [k, kt, c, w64:128]
            for cs in range(0, 128, 8):
                xt = []
                for ht in (0, 1):
                    t = xin.tile([128, 8, 128], BF16, tag=f"x{ht}")
                    nc.sync.dma_start(out=t, in_=xh_d[ht * 128:(ht + 1) * 128, cs:cs + 8, :])
                    nc.vector.tensor_reduce(out=xsum_h[:, ht, cs:cs + 8], in_=t,
                                            axis=AXX, op=ALU.add)
                    xt.append(t)
                for cc in (0, 4):
                    c0 = cs + cc
                    for kt in (0, 1):
                        pk = ps.tile([128, 4, 128], F32, tag=f"A{kt}")
                        for ht in (0, 1):
                            nc.tensor.matmul(pk, lhsT=tfwd_t[:, ht, kt * 128:(kt + 1) * 128],
                                             rhs=xt[ht][:, cc:cc + 4, :],
                                             start=(ht == 0), stop=(ht == 1))
                        for wh, ub in ((0, u0), (1, u1)):
                            nc.vector.tensor_tensor(
                                out=ub[:, kt, c0:c0 + 4, :],
                                in0=pk[:, :, wh * 64:(wh + 1) * 64],
                                in1=ghh[:, kt, c0:c0 + 4].unsqueeze(2).broadcast_to([128, 4, HQ]),
                                op=ALU.mult)

            # ---------------- HC inverse: y_h[c, h, w] ----------------
            y_h = ybuf.tile([128, 256, 128], BF16, tag="yh")
            for w in range(128):
                ut = u0 if w < HQ else u1
                pk = ps.tile([128, 256], F32, tag=f"B{w % 2}")
                for kt in (0, 1):
                    nc.tensor.matmul(pk, lhsT=ut[:, kt, :, w % HQ], rhs=tinv_t[:, kt, :],
                                     start=(kt == 0), stop=(kt == 1))
                if w % 2 == 0:
                    nc.vector.tensor_copy(y_h[:, :, w], pk)
                else:
                    nc.scalar.activation(y_h[:, :, w], pk, AF.Copy)

            # pool_w = sum_k sigw[k] ghw[k, c] * (T @ xsum_w)[k, c]  -> AllReduce #1
            # (all non-matmul work on gpsimd: vector/scalar queues must not
            #  block on the early-pass/AR dependency chain)
            xsum_wb = gsb.tile([128, 2, 128], BF16, tag="xsumwb")
            nc.gpsimd.tensor_copy(xsum_wb, xsum_w)
            t1 = []
            for kt in (0, 1):
                m1 = ps.tile([128, 128], F32, tag=f"B{kt}")
                for wt in (0, 1):
                    nc.tensor.matmul(m1, lhsT=tfwd_t[:, wt, kt * 128:(kt + 1) * 128],
                                     rhs=xsum_wb[:, wt, :], start=(wt == 0), stop=(wt == 1))
                tt = gsb.tile([128, 128], F32, tag=f"t1_{kt}")
                nc.vector.tensor_tensor(out=tt, in0=m1, in1=ghw[:, kt, :], op=ALU.mult)
                t1.append(tt)
            pw_ps = ps.tile([128, 1], F32, tag="A0")
            for kt in (0, 1):
                nc.tensor.matmul(pw_ps, lhsT=t1[kt], rhs=sigw_t[:, kt:kt + 1],
                                 start=(kt == 0), stop=(kt == 1))
            poolw_sb = gsb.tile([128, 1], F32, tag="poolw")
            nc.vector.tensor_copy(poolw_sb, pw_ps)
            nc.gpsimd.dma_start(out=arw_in[:], in_=poolw_sb)
            nc.gpsimd.collective_compute(
                "AllReduce", ALU.add,
                replica_groups=[[0, 1], [2, 3], [4, 5], [6, 7]],
                ins=[arw_in[:]], outs=[arw_out[:]])

            # pool_h = ghh[0, :] * sum_{h,w} xh  -> AllReduce #2
            ph_ps = ps.tile([128, 1], F32, tag="A1")
            for ht in (0, 1):
                nc.tensor.matmul(ph_ps, lhsT=xsum_h[:, ht, :], rhs=ones_t,
                                 start=(ht == 0), stop=(ht == 1))
            g0_ps = ps.tile([128, 1], F32, tag="B0")
            nc.tensor.matmul(g0_ps, lhsT=ghh[0:1, 0, :], rhs=one1_t, start=True, stop=True)
            g0_sb = gsb.tile([128, 1], F32, tag="g0sb")
            nc.vector.tensor_copy(g0_sb, g0_ps)
            poolh_sb = gsb.tile([128, 1], F32, tag="poolh")
            nc.vector.tensor_tensor(out=poolh_sb, in0=ph_ps, in1=g0_sb, op=ALU.mult)
            nc.gpsimd.dma_start(out=arh_in[:], in_=poolh_sb)
            nc.gpsimd.collective_compute(
                "AllReduce", ALU.add,
                replica_groups=[[0, 1], [2, 3], [4, 5], [6, 7]],
                ins=[arh_in[:]], outs=[arh_out[:]])

            wsc = consts.tile([128, 2, 256], BF16, tag="wsc")

            def attention():
                """AR results -> sigmoid scale -> fold into conv weights.

                Emitted after WC-q0 compute so the AllReduce latency hides
                behind it.  Elementwise ops go on gpsimd (its queue is the
                one already waiting on the ARs)."""
                p_sb = []
                for ct, aro in ((0, arh_out), (1, arw_out)):
                    pt = gsb.tile([128, 1], F32, tag=f"p_ar{ct}")
                    nc.gpsimd.dma_start(out=pt, in_=aro[:])
                    p_sb.append(pt)
                q_sb = []
                for ot in (0, 1):
                    q_ps = ps.tile([128, 1], F32, tag=f"B{ot}")
                    for ct in (0, 1):
                        nc.tensor.matmul(q_ps, lhsT=caw1t_t[:, ct, ot * 128:(ot + 1) * 128],
                                         rhs=p_sb[ct], start=(ct == 0), stop=(ct == 1))
                    qt = gsb.tile([128, 1], F32, tag=f"q{ot}")
                    nc.scalar.activation(qt, q_ps, AF.Gelu, bias=vec_t["cab1"][:, ot:ot + 1])
                    nc.gpsimd.tensor_tensor(out=qt, in0=qt, in1=vec_t["dwc"][:, ot:ot + 1],
                                            op=ALU.mult)
                    q_sb.append(qt)
                for ot in (0, 1):
                    s_t = gsb.tile([128, 1], F32, tag=f"s{ot}")
                    nc.scalar.activation(s_t, q_sb[ot], AF.Sigmoid,
                                         bias=vec_t["dwb"][:, ot:ot + 1])
                    nc.gpsimd.tensor_scalar_mul(wsc[:, ot, :], lcwt_t[:, ot, :], s_t)

            # ------- WC branch in h-quarters, conv fused per quarter -------
            for q in range(4):
                ub = (u0, u1)[q % 2]
                ywq = ybuf.tile([128, HQ, 128], BF16, tag=f"yw{q % 2}")
                # forward + gate
                for cs in range(0, 128, 8):
                    xt = []
                    for wt in (0, 1):
                        t = xin.tile([128, 8, HQ], BF16, tag=f"w{wt}")
                        nc.sync.dma_start(out=t, in_=xw_d[q, wt, :, cs:cs + 8, :])
                        xt.append(t)
                    for cc in (0, 4):
                        c0 = cs + cc
                        for kt in (0, 1):
                            pk = ps.tile([128, 4, HQ], F32, tag=f"A{kt}")
                            for wt in (0, 1):
                                nc.tensor.matmul(pk,
                                                 lhsT=tfwd_t[:, wt, kt * 128:(kt + 1) * 128],
                                                 rhs=xt[wt][:, cc:cc + 4, :],
                                                 start=(wt == 0), stop=(wt == 1))
                            nc.vector.tensor_tensor(
                                out=ub[:, kt, c0:c0 + 4, :],
                                in0=pk,
                                in1=ghw[:, kt, c0:c0 + 4].unsqueeze(2).broadcast_to([128, 4, HQ]),
                                op=ALU.mult)
                # inverse: y_w[c, hq, w] (local w columns)
                for hq in range(HQ):
                    pk = ps.tile([128, 128], F32, tag=f"B{hq % 2}")
                    for kt in (0, 1):
                        nc.tensor.matmul(pk, lhsT=ub[:, kt, :, hq], rhs=tinvw_t[:, kt, :],
                                         start=(kt == 0), stop=(kt == 1))
                    if hq % 2 == 0:
                        nc.vector.tensor_copy(ywq[:, hq, :], pk)
                    else:
                        nc.scalar.activation(ywq[:, hq, :], pk, AF.Copy)
                if q == 0:
                    attention()
                # conv 1x1 + BN + GELU + residual + store
                for hb in range(0, HQ, 4):
                    h0 = q * HQ + hb
                    for ot in (0, 1):
                        xrt = stg.tile([128, 4, 128], BF16, tag=f"xr{ot}")
                        nc.scalar.dma_start(out=xrt,
                                            in_=xres_d[ot * 128:(ot + 1) * 128, h0:h0 + 4, :])
                        gstg = stg.tile([128, 4, 128], BF16, tag=f"g{ot}")
                        po = ps.tile([128, 4, 128], F32, tag=f"A{ot}")
                        nc.tensor.matmul(po, lhsT=wsc[:, 0, ot * 128:(ot + 1) * 128],
                                         rhs=y_h[:, h0:h0 + 4, :],
                                         start=True, stop=False)
                        nc.tensor.matmul(po, lhsT=wsc[:, 1, ot * 128:(ot + 1) * 128],
                                         rhs=ywq[:, hb:hb + 4, :],
                                         start=False, stop=True)
                        nc.scalar.activation(gstg, po, AF.Gelu,
                                             bias=bnbeff[:, ot:ot + 1],
                                             scale=bninv[:, ot:ot + 1])
                        nc.gpsimd.tensor_add(gstg, gstg, xrt)
                        nc.gpsimd.dma_start(out=out_d[ot * 128:(ot + 1) * 128, h0:h0 + 4, :],
                                            in_=gstg)

    nc.compile()
    return nc


_NC_CACHE = None


def _get_nc():
    global _NC_CACHE
    if _NC_CACHE is None:
        _NC_CACHE = _build()
    return _NC_CACHE


def _host_consts(inputs, core):
    """Per-core constant inputs (everything except the x shards)."""
    s = core % 2
    wlo = WS * s
    T = _dft_basis()
    d = {}
    d["tfwd"] = _part_major(np.ascontiguousarray(T.T)).astype(_BF16_NP)
    d["tinv"] = _part_major(T).astype(_BF16_NP)
    d["tinvw"] = _part_major(np.ascontiguousarray(T[:, wlo:wlo + WS])).astype(_BF16_NP)
    d["sigw"] = _part_major(T[:, wlo:wlo + WS].sum(axis=1)).astype(np.float32)
    d["omega"] = (np.arange(129, dtype=np.float32) / 128.0 - 1.0).reshape(1, 129)
    d["lam"] = np.linspace(-1.0, 1.0, 128, dtype=np.float32).reshape(1, 128)
    for m in _MLPS:
        d[f"{m}_w1t"] = np.ascontiguousarray(inputs[f"{m}_w1"].T).astype(np.float32)
        d[f"{m}_b1v"] = inputs[f"{m}_b1"].reshape(64, 1).astype(np.float32)
        d[f"{m}_w2t"] = np.ascontiguousarray(inputs[f"{m}_w2"].T).astype(np.float32)
        d[f"{m}_b2v"] = inputs[f"{m}_b2"].reshape(64, 1).astype(np.float32)
        d[f"{m}_w3t"] = np.ascontiguousarray(inputs[f"{m}_w3"].T).astype(np.float32)
        d[f"{m}_b3v"] = inputs[f"{m}_b3"].reshape(8, 1).astype(np.float32)
    d["caw1t"] = _part_major(np.ascontiguousarray(inputs["ca_w1"].T) / 65536.0).astype(np.float32)
    d["cab1"] = _part_major(inputs["ca_b1"]).astype(np.float32)
    d["dwc"] = _part_major(np.ascontiguousarray(inputs["ca_dw"][:, 1, 1])).astype(np.float32)
    d["dwb"] = _part_major(inputs["ca_db"]).astype(np.float32)
    d["lcwt"] = _part_major(np.ascontiguousarray(inputs["lc_w"].T)).astype(_BF16_NP)
    d["bng"] = _part_major(inputs["bn_g"]).astype(np.float32)
    d["bnb"] = _part_major(inputs["bn_b"]).astype(np.float32)
    d["bnm"] = _part_major(inputs["bn_m"]).astype(np.float32)
    d["bnv"] = _part_major(inputs["bn_v"]).astype(np.float32)
    return d


def kernel(**inputs):
    x = np.asarray(inputs["x"], np.float32)
    nc = _get_nc()

    in_maps = []
    for core in range(NCORES):
        b, s = core // 2, core % 2
        wlo = WS * s
        m = _host_consts(inputs, core)
        m["xh"] = np.ascontiguousarray(
            x[b, :C2, :, wlo:wlo + WS].transpose(1, 0, 2)).astype(_BF16_NP)
        # [4q, 2wt, 128w, 128c, 64hq]
        xwq = x[b, C2:, :, :].reshape(C2, 4, HQ, N)
        m["xw"] = np.ascontiguousarray(
            xwq.transpose(1, 3, 0, 2).reshape(4, 2, 128, C2, HQ)).astype(_BF16_NP)
        m["xres"] = np.ascontiguousarray(x[b, :, :, wlo:wlo + WS]).astype(_BF16_NP)
        in_maps.append(m)

    trace = os.environ.get("BASS_KERNEL_TRACE", "0") == "1"
    res = bass_utils.run_bass_kernel_spmd(
        nc, in_maps, core_ids=list(range(NCORES)),
        trace=trace, trace_cores=list(range(NCORES)) if trace else None,
        stitch_traces=False)
    if trace and res.exec_time_ns is not None:
        print(f"HW exec time: {res.exec_time_ns} ns")
        print(f"   mean exec time: {res.mean_exec_time_ns} ns  "
              f"(slowest core {res.max_exec_time_core_id})")
        if res.instructions_and_trace is not None:
            print("   trace:", res.instructions_and_trace[1])

    out = np.empty((B, 2 * C2, N, N), np.float32)
    for core in range(NCORES):
        b, s = core // 2, core % 2
        wlo = WS * s
        out[b, :, :, wlo:wlo + WS] = res.results[core]["out"].astype(np.float32)
    return out


# revision 17
# speedup vs baseline: 1.0991x; 1.0983x over previous
"""Trainium2 Bass kernel for the spectral-gating network (nn_DAPSO).

Model (B=4, C=256, H=W=256):
  - channels 0:128   : y_h = irfft(Gh * rfft(x, axis=H))   (per-channel gate)
  - channels 128:256 : y_w = irfft(Gw * rfft(x, axis=W))
  - gates Gh/Gw from tiny MLPs (computed on device)
  - channel attention: s = sigmoid(dw(gelu(W1 @ mean_hw(y) + b)))  -> y *= s
  - y2 = gelu(BN(lc_w @ y));  out = x + y2

irfft(G*rfft(x)) along an axis of length N equals T^T diag(ghat) T x with T
the orthonormal real DFT basis, so both branches are dense TensorE matmuls.

Sharding: 8 cores = 4 batches x 2 w-halves; only cross-core traffic is two
1KB AllReduces of pooled channel means.

v3 (all-SBUF dataflow, AR-latency-free ordering): the inverse transforms are
data-stationary matmuls (lhsT = gated-spectrum column, rhs = inverse basis),
which lands branch outputs channel-major in SBUF directly -- no DRAM scratch.
The pooled means come straight from the gated spectrum u:
    sum_h y_w = sum_k sigw[k] * sum_h u_w[k,c,h]      (free-axis reduce of u)
    sum_{h,w} y_h = sqrt(N) * sum_w u_h[0,c,w]        (DC row of u)
so no separate input-sum pass exists.  Phase order WC -> HC -> conv puts
each AllReduce ~35-100us of compute ahead of its consumer.  The residual is
added on-chip; output is written once, in bf16.  Per-core HBM traffic:
~41MB read + 16MB written.
"""
import sys
import os

sys.path.insert(0, "/opt/trn_rl_repo")

import numpy as np
import ml_dtypes

import concourse.bacc as bacc
import concourse.mybir as mybir
import concourse.tile as tile
from concourse import bass_utils

F32 = mybir.dt.float32
BF16 = mybir.dt.bfloat16
AF = mybir.ActivationFunctionType
ALU = mybir.AluOpType
AXX = mybir.AxisListType.X

N = 256          # H = W
C2 = 128         # channels per branch
B = 4
NCORES = 8
WS = 128         # per-core w-slice width
HQ = 64          # WC h-quarter / HC w-half-of-half size

_BF16_NP = ml_dtypes.bfloat16


def _dft_basis():
    """Orthonormal real DFT basis T (N, N): y = T^T diag(ghat) T x == irfft(G*rfft(x))."""
    n = np.arange(N)
    k = np.arange(1, N // 2)
    T = np.zeros((N, N), np.float64)
    T[0, :] = 1.0 / np.sqrt(N)
    T[1:N // 2, :] = np.sqrt(2.0 / N) * np.cos(2 * np.pi * k[:, None] * n[None, :] / N)
    T[N // 2, :] = (1.0 / np.sqrt(N)) * ((-1.0) ** n)
    T[N // 2 + 1:, :] = np.sqrt(2.0 / N) * np.sin(2 * np.pi * k[:, None] * n[None, :] / N)
    return T.astype(np.float32)


def _part_major(a):
    """(256, ...) -> (128, 2, ...) partition-major layout."""
    a = np.asarray(a)
    return np.ascontiguousarray(a.reshape(2, 128, *a.shape[1:]).transpose(
        (1, 0) + tuple(range(2, a.ndim + 1))))


_MLPS = ("ah", "bc1", "aw", "bc2")


def _build():
    nc = bacc.Bacc("TRN2", target_bir_lowering=False, num_devices=NCORES)

    # ---------------- I/O declarations ----------------
    # xh: [2ws, 2ht, 128h, 128c, 64w]   (HC input, h on partitions)
    xh_d = nc.dram_tensor("xh", [2, 2, 128, 128, HQ], BF16, kind="ExternalInput")
    # xw: [4q, 2wt, 128w, 128c, 64h]    (WC input, w on partitions)
    xw_d = nc.dram_tensor("xw", [4, 2, 128, 128, HQ], BF16, kind="ExternalInput")
    xres_d = nc.dram_tensor("xres", [256, 256, 128], BF16, kind="ExternalInput")
    tfwd_d = nc.dram_tensor("tfwd", [128, 2, 256], BF16, kind="ExternalInput")
    tinv_d = nc.dram_tensor("tinv", [128, 2, 256], BF16, kind="ExternalInput")
    tinvw_d = nc.dram_tensor("tinvw", [128, 2, 128], BF16, kind="ExternalInput")
    sigw_d = nc.dram_tensor("sigw", [128, 2], F32, kind="ExternalInput")
    omega_d = nc.dram_tensor("omega", [1, 129], F32, kind="ExternalInput")
    lam_d = nc.dram_tensor("lam", [1, 128], F32, kind="ExternalInput")
    mlp_d = {}
    for m in _MLPS:
        mlp_d[m] = dict(
            w1t=nc.dram_tensor(f"{m}_w1t", [1, 64], F32, kind="ExternalInput"),
            b1=nc.dram_tensor(f"{m}_b1v", [64, 1], F32, kind="ExternalInput"),
            w2t=nc.dram_tensor(f"{m}_w2t", [64, 64], F32, kind="ExternalInput"),
            b2=nc.dram_tensor(f"{m}_b2v", [64, 1], F32, kind="ExternalInput"),
            w3t=nc.dram_tensor(f"{m}_w3t", [64, 8], F32, kind="ExternalInput"),
            b3=nc.dram_tensor(f"{m}_b3v", [8, 1], F32, kind="ExternalInput"),
        )
    caw1t_d = nc.dram_tensor("caw1t", [128, 2, 256], F32, kind="ExternalInput")
    cab1_d = nc.dram_tensor("cab1", [128, 2], F32, kind="ExternalInput")
    dwc_d = nc.dram_tensor("dwc", [128, 2], F32, kind="ExternalInput")
    dwb_d = nc.dram_tensor("dwb", [128, 2], F32, kind="ExternalInput")
    lcwt_d = nc.dram_tensor("lcwt", [128, 2, 256], BF16, kind="ExternalInput")
    bng_d = nc.dram_tensor("bng", [128, 2], F32, kind="ExternalInput")
    bnb_d = nc.dram_tensor("bnb", [128, 2], F32, kind="ExternalInput")
    bnm_d = nc.dram_tensor("bnm", [128, 2], F32, kind="ExternalInput")
    bnv_d = nc.dram_tensor("bnv", [128, 2], F32, kind="ExternalInput")

    out_d = nc.dram_tensor("out", [256, 256, 128], BF16, kind="ExternalOutput")

    arh_in = nc.dram_tensor("arh_in", [128, 1], F32)
    arh_out = nc.dram_tensor("arh_out", [128, 1], F32)
    arw_in = nc.dram_tensor("arw_in", [128, 1], F32)
    arw_out = nc.dram_tensor("arw_out", [128, 1], F32)

    with tile.TileContext(nc) as tc:
        with tc.tile_pool(name="consts", bufs=1) as consts, \
             tc.tile_pool(name="xin", bufs=2) as xin, \
             tc.tile_pool(name="ubuf", bufs=1) as ubuf, \
             tc.tile_pool(name="ybuf", bufs=1) as ybuf, \
             tc.tile_pool(name="stg", bufs=2) as stg, \
             tc.tile_pool(name="gsb", bufs=1) as gsb, \
             tc.tile_pool(name="psA", bufs=1, space="PSUM") as psA, \
             tc.tile_pool(name="psB", bufs=2, space="PSUM") as psB:

            def pa(i):
                return psA.tile([128, 8, HQ], F32, tag=f"A{i % 4}", name=f"pa{i % 4}")

            # ---------------- const loads ----------------
            tfwd_t = consts.tile([128, 2, 256], BF16, tag="tfwd")
            nc.sync.dma_start(out=tfwd_t, in_=tfwd_d[:])
            tinv_t = consts.tile([128, 2, 256], BF16, tag="tinv")
            nc.sync.dma_start(out=tinv_t, in_=tinv_d[:])
            tinvw_t = consts.tile([128, 2, 128], BF16, tag="tinvw")
            nc.sync.dma_start(out=tinvw_t, in_=tinvw_d[:])
            sigw_t = consts.tile([128, 2], F32, tag="sigw")
            nc.sync.dma_start(out=sigw_t, in_=sigw_d[:])
            caw1t_t = consts.tile([128, 2, 256], F32, tag="caw1t")
            nc.sync.dma_start(out=caw1t_t, in_=caw1t_d[:])
            lcwt_t = consts.tile([128, 2, 256], BF16, tag="lcwt")
            nc.sync.dma_start(out=lcwt_t, in_=lcwt_d[:])
            vec_t = {}
            for nm, d in (("cab1", cab1_d), ("dwc", dwc_d), ("dwb", dwb_d),
                          ("bng", bng_d), ("bnb", bnb_d), ("bnm", bnm_d), ("bnv", bnv_d)):
                vt = consts.tile([128, 2], F32, tag=f"v_{nm}")
                nc.sync.dma_start(out=vt, in_=d[:])
                vec_t[nm] = vt
            omega_t = consts.tile([1, 129], F32, tag="omega")
            nc.sync.dma_start(out=omega_t, in_=omega_d[:])
            lam_t = consts.tile([1, 128], F32, tag="lam")
            nc.sync.dma_start(out=lam_t, in_=lam_d[:])
            one16_t = consts.tile([1, 1], F32, tag="one16")
            nc.vector.memset(one16_t, float(np.sqrt(N)))

            # ---------------- gate MLPs (tiny), table-batched ----------------
            def mlp_head(m, xvec, nk, role):
                d = mlp_d[m]
                w1t = gsb.tile([1, 64], F32, tag="m_w1")
                nc.sync.dma_start(out=w1t, in_=d["w1t"][:])
                b1 = gsb.tile([64, 1], F32, tag="m_b1")
                nc.sync.dma_start(out=b1, in_=d["b1"][:])
                w2t = gsb.tile([64, 64], F32, tag="m_w2")
                nc.sync.dma_start(out=w2t, in_=d["w2t"][:])
                b2 = gsb.tile([64, 1], F32, tag="m_b2")
                nc.sync.dma_start(out=b2, in_=d["b2"][:])
                w3t = gsb.tile([64, 8], F32, tag="m_w3")
                nc.sync.dma_start(out=w3t, in_=d["w3t"][:])
                b3 = gsb.tile([8, 1], F32, tag="m_b3")
                nc.sync.dma_start(out=b3, in_=d["b3"][:])

                p1 = psB.tile([64, nk], F32, tag="B0")
                nc.tensor.matmul(p1, lhsT=w1t, rhs=xvec, start=True, stop=True)
                h1 = gsb.tile([64, nk], F32, tag="m_h1")
                nc.scalar.activation(h1, p1, AF.Gelu, bias=b1)
                p2 = psB.tile([64, nk], F32, tag="B1")
                nc.tensor.matmul(p2, lhsT=w2t, rhs=h1, start=True, stop=True)
                h2 = gsb.tile([64, nk], F32, tag="m_h2")
                nc.scalar.activation(h2, p2, AF.Gelu, bias=b2)
                p3 = psB.tile([8, nk], F32, tag="B0")
                nc.tensor.matmul(p3, lhsT=w3t, rhs=h2, start=True, stop=True)
                at = gsb.tile([8, nk], F32, tag=f"m_at{role}")
                nc.scalar.activation(at, p3, AF.Identity, bias=b3)
                return at

            ghh = consts.tile([128, 2, 128], F32, tag="ghh")
            ghw = consts.tile([128, 2, 128], F32, tag="ghw")
            gp = {}
            gtags = {("h", 0): "A0", ("h", 1): "A1", ("w", 0): "A2", ("w", 1): "A3"}
            for (am, bm, nmk) in (("aw", "bc2", "w"), ("ah", "bc1", "h")):
                at = mlp_head(am, omega_t, 129, "a")
                bt = mlp_head(bm, lam_t, 128, "b")
                g0 = psA.tile([128, 128], F32, tag=gtags[(nmk, 0)])
                nc.tensor.matmul(g0, lhsT=at[:, 0:128], rhs=bt, start=True, stop=True)
                gn = psA.tile([1, 128], F32, tag=gtags[(nmk, 1)])
                nc.tensor.matmul(gn, lhsT=at[:, 128:129], rhs=bt, start=True, stop=True)
                gp[(nmk, 0)] = g0
                gp[(nmk, 1)] = gn
            # softplus(z) = relu(z) + log1p(exp(-|z|)), stage-batched across all 4
            keys = list(gp.keys())
            sp = {}
            for i, key in enumerate(keys):
                npart = 128 if key[1] == 0 else 1
                na = gsb.tile([128, 128], F32, tag=f"sp_na{i}")
                nc.scalar.activation(na[:npart, :], gp[key], AF.Abs)
                sp[key] = na
            for i, key in enumerate(keys):
                npart = 128 if key[1] == 0 else 1
                ex = gsb.tile([128, 128], F32, tag=f"sp_ex{i}")
                nc.scalar.activation(ex[:npart, :], sp[key][:npart, :], AF.Exp, scale=-1.0)
                nc.vector.tensor_scalar_add(ex[:npart, :], ex[:npart, :], 1.0)
                sp[key] = ex
            for key in keys:
                npart = 128 if key[1] == 0 else 1
                nc.scalar.activation(sp[key][:npart, :], sp[key][:npart, :], AF.Ln)
            for i, key in enumerate(keys):
                npart = 128 if key[1] == 0 else 1
                re = gsb.tile([128, 128], F32, tag=f"sp_na{i}")
                nc.scalar.activation(re[:npart, :], gp[key], AF.Relu)
                gh = ghh if key[0] == "h" else ghw
                if key[1] == 0:
                    nc.vector.tensor_add(gh[:, 0, :], sp[key][:128, :], re[:128, :])
                else:
                    # rows 128+j of ghat equal G[j]: copy the aligned block first,
                    # then overwrite row 0 with the Nyquist G[128].
                    nc.vector.tensor_copy(gh[:, 1, :], gh[:, 0, :])
                    nc.vector.tensor_add(gh[0:1, 1, :], sp[key][0:1, :], re[0:1, :])
            for gh in (ghh, ghw):
                nc.vector.tensor_scalar_mul(gh[:, :, :], gh[:, :, :], float(8.0 ** -0.5))

            # ---------------- BN prep ----------------
            bninv = consts.tile([128, 2], F32, tag="bninv")
            nc.vector.tensor_scalar_add(bninv, vec_t["bnv"], 1e-5)
            nc.scalar.activation(bninv, bninv, AF.Sqrt)
            nc.vector.reciprocal(bninv, bninv)
            nc.vector.tensor_tensor(out=bninv, in0=vec_t["bng"], in1=bninv, op=ALU.mult)
            bnbeff = consts.tile([128, 2], F32, tag="bnbeff")
            nc.vector.tensor_tensor(out=bnbeff, in0=vec_t["bnm"], in1=bninv, op=ALU.mult)
            nc.vector.tensor_tensor(out=bnbeff, in0=vec_t["bnb"], in1=bnbeff, op=ALU.subtract)

            u_t = ubuf.tile([128, 2, 128, HQ], BF16, tag="u")    # [k, kt, c, hq|w]
            usum = consts.tile([128, 2, 128], F32, tag="usum")   # [k, kt, c]
            phrow = [gsb.tile([1, 128], F32, tag=f"phrow{i}", name=f"phrow{i}")
                     for i in (0, 1)]

            def fwd_quarter(x_dram, idx, gate, nseg):
                """Load one 16c-chunked segment set, forward-transform, gate
                into u_t.  idx selects [idx] of x_dram's leading dim pair."""
                nmm = 0
                for cs in range(0, 128, 16):
                    xt = []
                    for t2 in (0, 1):
                        t = xin.tile([128, 16, HQ], BF16, tag=f"x{t2}")
                        nc.sync.dma_start(out=t, in_=x_dram[idx, t2, :, cs:cs + 16, :])
                        xt.append(t)
                    for cc in (0, 8):
                        c0 = cs + cc
                        for kt in (0, 1):
                            pk = pa(nmm)
                            nmm += 1
                            for t2 in (0, 1):
                                nc.tensor.matmul(pk,
                                                 lhsT=tfwd_t[:, t2, kt * 128:(kt + 1) * 128],
                                                 rhs=xt[t2][:, cc:cc + 8, :],
                                                 start=(t2 == 0), stop=(t2 == 1))
                            nc.vector.tensor_tensor(
                                out=u_t[:, kt, c0:c0 + 8, :],
                                in0=pk,
                                in1=gate[:, kt, c0:c0 + 8].unsqueeze(2)
                                    .broadcast_to([128, 8, HQ]),
                                op=ALU.mult)

            # =========== WC branch in h-quarters ===========
            y_w = [ybuf.tile([128, HQ, 128], BF16, tag=f"yw{q}", name=f"yw{q}")
                   for q in range(4)]
            for q in range(4):
                fwd_quarter(xw_d, q, ghw, 2)
                # pooled-mean accumulation: usum += sum_h u
                if q == 0:
                    nc.vector.tensor_reduce(out=usum, in_=u_t, axis=AXX, op=ALU.add)
                else:
                    ur = gsb.tile([128, 2, 128], F32, tag="ur")
                    nc.vector.tensor_reduce(out=ur, in_=u_t, axis=AXX, op=ALU.add)
                    nc.vector.tensor_add(usum, usum, ur)
                if q == 3:
                    # pool_w[c] = sum_k sigw[k] * usum[k, c]  -> AllReduce #1
                    pw_ps = psB.tile([128, 1], F32, tag="B0")
                    for kt in (0, 1):
                        nc.tensor.matmul(pw_ps, lhsT=usum[:, kt, :],
                                         rhs=sigw_t[:, kt:kt + 1],
                                         start=(kt == 0), stop=(kt == 1))
                    poolw_sb = gsb.tile([128, 1], F32, tag="poolw")
                    nc.vector.tensor_copy(poolw_sb, pw_ps)
                    nc.gpsimd.dma_start(out=arw_in[:], in_=poolw_sb)
                    nc.gpsimd.collective_compute(
                        "AllReduce", ALU.add,
                        replica_groups=[[0, 1], [2, 3], [4, 5], [6, 7]],
                        ins=[arw_in[:]], outs=[arw_out[:]])
                # inverse: y_w[c, hq, w], two h per PSUM tile, one drain each
                for hp in range(0, HQ, 2):
                    pk = psB.tile([128, 2, 128], F32, tag=f"B{(hp // 2) % 2}")
                    for i in (0, 1):
                        for kt in (0, 1):
                            nc.tensor.matmul(pk[:, i, :], lhsT=u_t[:, kt, :, hp + i],
                                             rhs=tinvw_t[:, kt, :],
                                             start=(kt == 0), stop=(kt == 1))
                    if (hp // 2) % 2 == 0:
                        nc.vector.tensor_copy(y_w[q][:, hp:hp + 2, :], pk)
                    else:
                        nc.scalar.activation(y_w[q][:, hp:hp + 2, :], pk, AF.Copy)

            # =========== HC branch in w-halves ===========
            y_h = ybuf.tile([128, 256, 128], BF16, tag="yh")
            for ws in (0, 1):
                fwd_quarter(xh_d, ws, ghh, 2)
                # DC-row trick: sum_{h,w} y_h = sqrt(N) * sum_w u[0, c, w]
                nc.vector.tensor_reduce(out=phrow[ws], in_=u_t[0:1, 0, :, :],
                                        axis=AXX, op=ALU.add)
                if ws == 1:
                    nc.vector.tensor_add(phrow[0], phrow[0], phrow[1])
                    ph_ps = psB.tile([128, 1], F32, tag="B0")
                    nc.tensor.matmul(ph_ps, lhsT=phrow[0], rhs=one16_t,
                                     start=True, stop=True)
                    poolh_sb = gsb.tile([128, 1], F32, tag="poolh")
                    nc.vector.tensor_copy(poolh_sb, ph_ps)
                    nc.gpsimd.dma_start(out=arh_in[:], in_=poolh_sb)
                    nc.gpsimd.collective_compute(
                        "AllReduce", ALU.add,
                        replica_groups=[[0, 1], [2, 3], [4, 5], [6, 7]],
                        ins=[arh_in[:]], outs=[arh_out[:]])
                # inverse: y_h[c, h, w] for local w columns ws*64 + [0, 64)
                for w in range(HQ):
                    wl = ws * HQ + w
                    pk = psB.tile([128, 256], F32, tag=f"B{w % 2}")
                    for kt in (0, 1):
                        nc.tensor.matmul(pk, lhsT=u_t[:, kt, :, w],
                                         rhs=tinv_t[:, kt, :],
                                         start=(kt == 0), stop=(kt == 1))
                    if w % 2 == 0:
                        nc.vector.tensor_copy(y_h[:, :, wl], pk)
                    else:
                        nc.scalar.activation(y_h[:, :, wl], pk, AF.Copy)

            # ---------------- channel attention -> folded conv weights --------
            wsc = consts.tile([128, 2, 256], BF16, tag="wsc")
            p_sb = []
            for ct, aro in ((0, arh_out), (1, arw_out)):
                pt = gsb.tile([128, 1], F32, tag=f"p_ar{ct}")
                nc.gpsimd.dma_start(out=pt, in_=aro[:])
                p_sb.append(pt)
            q_sb = []
            for ot in (0, 1):
                q_ps = psB.tile([128, 1], F32, tag=f"B{ot}")
                for ct in (0, 1):
                    nc.tensor.matmul(q_ps, lhsT=caw1t_t[:, ct, ot * 128:(ot + 1) * 128],
                                     rhs=p_sb[ct], start=(ct == 0), stop=(ct == 1))
                qt = gsb.tile([128, 1], F32, tag=f"q{ot}")
                nc.scalar.activation(qt, q_ps, AF.Gelu, bias=vec_t["cab1"][:, ot:ot + 1])
                nc.gpsimd.tensor_tensor(out=qt, in0=qt, in1=vec_t["dwc"][:, ot:ot + 1],
                                        op=ALU.mult)
                q_sb.append(qt)
            for ot in (0, 1):
                s_t = gsb.tile([128, 1], F32, tag=f"s{ot}")
                nc.scalar.activation(s_t, q_sb[ot], AF.Sigmoid,
                                     bias=vec_t["dwb"][:, ot:ot + 1])
                nc.gpsimd.tensor_scalar_mul(wsc[:, ot, :], lcwt_t[:, ot, :], s_t)

            # ---------------- conv 1x1 + BN + GELU + residual + store ---------
            nmm = 0
            for hc in range(0, 256, 8):
                ywq = y_w[hc // HQ]
                hof = hc % HQ
                for ot in (0, 1):
                    xrt = stg.tile([128, 8, 128], BF16, tag=f"xr{ot}")
                    dq = nc.scalar if ot == 0 else nc.sync
                    dq.dma_start(out=xrt,
                                 in_=xres_d[ot * 128:(ot + 1) * 128, hc:hc + 8, :])
                    ostg = stg.tile([128, 8, 128], BF16, tag=f"g{ot}")
                    for sl in (0, 4):
                        po = psA.tile([128, 4, 128], F32, tag=f"A{nmm % 4}")
                        nmm += 1
                        nc.tensor.matmul(po, lhsT=wsc[:, 0, ot * 128:(ot + 1) * 128],
                                         rhs=y_h[:, hc + sl:hc + sl + 4, :],
                                         start=True, stop=False)
                        nc.tensor.matmul(po, lhsT=wsc[:, 1, ot * 128:(ot + 1) * 128],
                                         rhs=ywq[:, hof + sl:hof + sl + 4, :],
                                         start=False, stop=True)
                        nc.scalar.activation(ostg[:, sl:sl + 4, :], po, AF.Gelu,
                                             bias=bnbeff[:, ot:ot + 1],
                                             scale=bninv[:, ot:ot + 1])
                    nc.gpsimd.tensor_add(ostg, ostg, xrt)
                    nc.gpsimd.dma_start(out=out_d[ot * 128:(ot + 1) * 128, hc:hc + 8, :],
                                        in_=ostg)

    nc.compile()
    return nc


_NC_CACHE = None


def _get_nc():
    global _NC_CACHE
    if _NC_CACHE is None:
        _NC_CACHE = _build()
    return _NC_CACHE


def _host_consts(inputs, core):
    """Per-core constant inputs (everything except the x shards)."""
    s = core % 2
    wlo = WS * s
    T = _dft_basis()
    d = {}
    d["tfwd"] = _part_major(np.ascontiguousarray(T.T)).astype(_BF16_NP)
    d["tinv"] = _part_major(T).astype(_BF16_NP)
    d["tinvw"] = _part_major(np.ascontiguousarray(T[:, wlo:wlo + WS])).astype(_BF16_NP)
    d["sigw"] = _part_major(T[:, wlo:wlo + WS].sum(axis=1)).astype(np.float32)
    d["omega"] = (np.arange(129, dtype=np.float32) / 128.0 - 1.0).reshape(1, 129)
    d["lam"] = np.linspace(-1.0, 1.0, 128, dtype=np.float32).reshape(1, 128)
    for m in _MLPS:
        d[f"{m}_w1t"] = np.ascontiguousarray(inputs[f"{m}_w1"].T).astype(np.float32)
        d[f"{m}_b1v"] = inputs[f"{m}_b1"].reshape(64, 1).astype(np.float32)
        d[f"{m}_w2t"] = np.ascontiguousarray(inputs[f"{m}_w2"].T).astype(np.float32)
        d[f"{m}_b2v"] = inputs[f"{m}_b2"].reshape(64, 1).astype(np.float32)
        d[f"{m}_w3t"] = np.ascontiguousarray(inputs[f"{m}_w3"].T).astype(np.float32)
        d[f"{m}_b3v"] = inputs[f"{m}_b3"].reshape(8, 1).astype(np.float32)
    d["caw1t"] = _part_major(np.ascontiguousarray(inputs["ca_w1"].T) / 65536.0).astype(np.float32)
    d["cab1"] = _part_major(inputs["ca_b1"]).astype(np.float32)
    d["dwc"] = _part_major(np.ascontiguousarray(inputs["ca_dw"][:, 1, 1])).astype(np.float32)
    d["dwb"] = _part_major(inputs["ca_db"]).astype(np.float32)
    d["lcwt"] = _part_major(np.ascontiguousarray(inputs["lc_w"].T)).astype(_BF16_NP)
    d["bng"] = _part_major(inputs["bn_g"]).astype(np.float32)
    d["bnb"] = _part_major(inputs["bn_b"]).astype(np.float32)
    d["bnm"] = _part_major(inputs["bn_m"]).astype(np.float32)
    d["bnv"] = _part_major(inputs["bn_v"]).astype(np.float32)
    return d


def kernel(**inputs):
    x = np.asarray(inputs["x"], np.float32)
    nc = _get_nc()

    in_maps = []
    for core in range(NCORES):
        b, s = core // 2, core % 2
        wlo = WS * s
        m = _host_consts(inputs, core)
        # xh: [2ws, 2ht, 128h, 128c, 64w]
        xhq = x[b, :C2, :, wlo:wlo + WS]             # (128c, 256h, 128w)
        m["xh"] = np.ascontiguousarray(
            xhq.reshape(C2, 2, 128, 2, HQ).transpose(3, 1, 2, 0, 4)).astype(_BF16_NP)
        # xw: [4q, 2wt, 128w, 128c, 64hq]
        xwq = x[b, C2:, :, :].reshape(C2, 4, HQ, N)
        m["xw"] = np.ascontiguousarray(
            xwq.transpose(1, 3, 0, 2).reshape(4, 2, 128, C2, HQ)).astype(_BF16_NP)
        m["xres"] = np.ascontiguousarray(x[b, :, :, wlo:wlo + WS]).astype(_BF16_NP)
        in_maps.append(m)

    trace = os.environ.get("BASS_KERNEL_TRACE", "0") == "1"
    res = bass_utils.run_bass_kernel_spmd(
        nc, in_maps, core_ids=list(range(NCORES)),
        trace=trace, trace_cores=list(range(NCORES)) if trace else None,
        stitch_traces=False)
    if trace and res.exec_time_ns is not None:
        print(f"HW exec time: {res.exec_time_ns} ns")
        print(f"   mean exec time: {res.mean_exec_time_ns} ns  "
              f"(slowest core {res.max_exec_time_core_id})")
        if res.instructions_and_trace is not None:
            print("   trace:", res.instructions_and_trace[1])

    out = np.empty((B, 2 * C2, N, N), np.float32)
    for core in range(NCORES):
        b, s = core // 2, core % 2
        wlo = WS * s
        out[b, :, :, wlo:wlo + WS] = res.results[core]["out"].astype(np.float32)
    return out


# revision 20
# speedup vs baseline: 1.3790x; 1.2546x over previous
"""Trainium2 Bass kernel for the spectral-gating network (nn_DAPSO).

Model (B=4, C=256, H=W=256):
  - channels 0:128   : y_h = irfft(Gh * rfft(x, axis=H))   (per-channel gate)
  - channels 128:256 : y_w = irfft(Gw * rfft(x, axis=W))
  - gates Gh/Gw from tiny MLPs (computed on device)
  - channel attention: s = sigmoid(dw(gelu(W1 @ mean_hw(y) + b)))  -> y *= s
  - y2 = gelu(BN(lc_w @ y));  out = x + y2

irfft(G*rfft(x)) along an axis of length N equals T^T diag(ghat) T x with T
the orthonormal real DFT basis, so both branches are dense TensorE matmuls.

Sharding: 8 cores = 4 batches x 2 w-halves; only cross-core traffic is two
1KB AllReduces of pooled channel means.

v4: all-SBUF dataflow (no DRAM scratch), AR-latency-free phase order
(WC -> HC -> conv), pooled means read off the gated spectrum u
(sum_h u for pool_w via gpsimd tree on the dead u buffer; DC row for
pool_h), data-stationary inverse transforms landing channel-major, and
big-op engine balance: 2-bank PSUM tiles, 16c forward chunks, 8h/4w
inverse drain groups, conv at 8h, residual add split vector/gpsimd.
Per-core HBM traffic: ~41MB read + 16MB written.
"""
import sys
import os

sys.path.insert(0, "/opt/trn_rl_repo")

import numpy as np
import ml_dtypes

import concourse.bacc as bacc
import concourse.mybir as mybir
import concourse.tile as tile
from concourse import bass_utils

F32 = mybir.dt.float32
BF16 = mybir.dt.bfloat16
AF = mybir.ActivationFunctionType
ALU = mybir.AluOpType
AXX = mybir.AxisListType.X

N = 256          # H = W
C2 = 128         # channels per branch
B = 4
NCORES = 8
WS = 128         # per-core w-slice width
HQ = 64          # WC h-quarter / HC w-half-of-half size

_BF16_NP = ml_dtypes.bfloat16


def _dft_basis():
    """Orthonormal real DFT basis T (N, N): y = T^T diag(ghat) T x == irfft(G*rfft(x))."""
    n = np.arange(N)
    k = np.arange(1, N // 2)
    T = np.zeros((N, N), np.float64)
    T[0, :] = 1.0 / np.sqrt(N)
    T[1:N // 2, :] = np.sqrt(2.0 / N) * np.cos(2 * np.pi * k[:, None] * n[None, :] / N)
    T[N // 2, :] = (1.0 / np.sqrt(N)) * ((-1.0) ** n)
    T[N // 2 + 1:, :] = np.sqrt(2.0 / N) * np.sin(2 * np.pi * k[:, None] * n[None, :] / N)
    return T.astype(np.float32)


def _part_major(a):
    """(256, ...) -> (128, 2, ...) partition-major layout."""
    a = np.asarray(a)
    return np.ascontiguousarray(a.reshape(2, 128, *a.shape[1:]).transpose(
        (1, 0) + tuple(range(2, a.ndim + 1))))


_MLPS = ("ah", "bc1", "aw", "bc2")


def _build():
    nc = bacc.Bacc("TRN2", target_bir_lowering=False, num_devices=NCORES)

    # ---------------- I/O declarations ----------------
    # xh: [2ws, 2ht, 128h, 128c, 64w]   (HC input, h on partitions)
    xh_d = nc.dram_tensor("xh", [2, 2, 128, 128, HQ], BF16, kind="ExternalInput")
    # xw: [4q, 2wt, 128w, 128c, 64h]    (WC input, w on partitions)
    xw_d = nc.dram_tensor("xw", [4, 2, 128, 128, HQ], BF16, kind="ExternalInput")
    xres_d = nc.dram_tensor("xres", [256, 256, 128], BF16, kind="ExternalInput")
    tfwd_d = nc.dram_tensor("tfwd", [128, 2, 256], BF16, kind="ExternalInput")
    tinv_d = nc.dram_tensor("tinv", [128, 2, 256], BF16, kind="ExternalInput")
    tinvw_d = nc.dram_tensor("tinvw", [128, 2, 128], BF16, kind="ExternalInput")
    sigw_d = nc.dram_tensor("sigw", [128, 2], F32, kind="ExternalInput")
    omega_d = nc.dram_tensor("omega", [1, 129], F32, kind="ExternalInput")
    lam_d = nc.dram_tensor("lam", [1, 128], F32, kind="ExternalInput")
    mlp_d = {}
    for m in _MLPS:
        mlp_d[m] = dict(
            w1t=nc.dram_tensor(f"{m}_w1t", [1, 64], F32, kind="ExternalInput"),
            b1=nc.dram_tensor(f"{m}_b1v", [64, 1], F32, kind="ExternalInput"),
            w2t=nc.dram_tensor(f"{m}_w2t", [64, 64], F32, kind="ExternalInput"),
            b2=nc.dram_tensor(f"{m}_b2v", [64, 1], F32, kind="ExternalInput"),
            w3t=nc.dram_tensor(f"{m}_w3t", [64, 8], F32, kind="ExternalInput"),
            b3=nc.dram_tensor(f"{m}_b3v", [8, 1], F32, kind="ExternalInput"),
        )
    caw1t_d = nc.dram_tensor("caw1t", [128, 2, 256], F32, kind="ExternalInput")
    cab1_d = nc.dram_tensor("cab1", [128, 2], F32, kind="ExternalInput")
    dwc_d = nc.dram_tensor("dwc", [128, 2], F32, kind="ExternalInput")
    dwb_d = nc.dram_tensor("dwb", [128, 2], F32, kind="ExternalInput")
    lcwt_d = nc.dram_tensor("lcwt", [128, 2, 256], BF16, kind="ExternalInput")
    bng_d = nc.dram_tensor("bng", [128, 2], F32, kind="ExternalInput")
    bnb_d = nc.dram_tensor("bnb", [128, 2], F32, kind="ExternalInput")
    bnm_d = nc.dram_tensor("bnm", [128, 2], F32, kind="ExternalInput")
    bnv_d = nc.dram_tensor("bnv", [128, 2], F32, kind="ExternalInput")

    out_d = nc.dram_tensor("out", [256, 256, 128], BF16, kind="ExternalOutput")

    arh_in = nc.dram_tensor("arh_in", [128, 1], F32)
    arh_out = nc.dram_tensor("arh_out", [128, 1], F32)
    arw_in = nc.dram_tensor("arw_in", [128, 1], F32)
    arw_out = nc.dram_tensor("arw_out", [128, 1], F32)

    with tile.TileContext(nc) as tc:
        with tc.tile_pool(name="consts", bufs=1) as consts, \
             tc.tile_pool(name="xin", bufs=2) as xin, \
             tc.tile_pool(name="ubuf", bufs=1) as ubuf, \
             tc.tile_pool(name="ybuf", bufs=1) as ybuf, \
             tc.tile_pool(name="stg", bufs=2) as stg, \
             tc.tile_pool(name="gsb", bufs=1) as gsb, \
             tc.tile_pool(name="psA", bufs=1, space="PSUM") as psA, \
             tc.tile_pool(name="psB", bufs=1, space="PSUM") as psB:

            # ---------------- const loads ----------------
            tfwd_t = consts.tile([128, 2, 256], BF16, tag="tfwd")
            nc.sync.dma_start(out=tfwd_t, in_=tfwd_d[:])
            tinv_t = consts.tile([128, 2, 256], BF16, tag="tinv")
            nc.sync.dma_start(out=tinv_t, in_=tinv_d[:])
            tinvw_t = consts.tile([128, 2, 128], BF16, tag="tinvw")
            nc.sync.dma_start(out=tinvw_t, in_=tinvw_d[:])
            sigw_t = consts.tile([128, 2], F32, tag="sigw")
            nc.sync.dma_start(out=sigw_t, in_=sigw_d[:])
            caw1t_t = consts.tile([128, 2, 256], F32, tag="caw1t")
            nc.sync.dma_start(out=caw1t_t, in_=caw1t_d[:])
            lcwt_t = consts.tile([128, 2, 256], BF16, tag="lcwt")
            nc.sync.dma_start(out=lcwt_t, in_=lcwt_d[:])
            vec_t = {}
            for nm, d in (("cab1", cab1_d), ("dwc", dwc_d), ("dwb", dwb_d),
                          ("bng", bng_d), ("bnb", bnb_d), ("bnm", bnm_d), ("bnv", bnv_d)):
                vt = consts.tile([128, 2], F32, tag=f"v_{nm}")
                nc.sync.dma_start(out=vt, in_=d[:])
                vec_t[nm] = vt
            omega_t = consts.tile([1, 129], F32, tag="omega")
            nc.sync.dma_start(out=omega_t, in_=omega_d[:])
            lam_t = consts.tile([1, 128], F32, tag="lam")
            nc.sync.dma_start(out=lam_t, in_=lam_d[:])
            one16_t = consts.tile([1, 1], F32, tag="one16")
            nc.vector.memset(one16_t, float(np.sqrt(N)))

            # ---------------- gate MLPs (tiny), table-batched ----------------
            def mlp_head(m, xvec, nk, role):
                d = mlp_d[m]
                w1t = gsb.tile([1, 64], F32, tag="m_w1")
                nc.sync.dma_start(out=w1t, in_=d["w1t"][:])
                b1 = gsb.tile([64, 1], F32, tag="m_b1")
                nc.sync.dma_start(out=b1, in_=d["b1"][:])
                w2t = gsb.tile([64, 64], F32, tag="m_w2")
                nc.sync.dma_start(out=w2t, in_=d["w2t"][:])
                b2 = gsb.tile([64, 1], F32, tag="m_b2")
                nc.sync.dma_start(out=b2, in_=d["b2"][:])
                w3t = gsb.tile([64, 8], F32, tag="m_w3")
                nc.sync.dma_start(out=w3t, in_=d["w3t"][:])
                b3 = gsb.tile([8, 1], F32, tag="m_b3")
                nc.sync.dma_start(out=b3, in_=d["b3"][:])

                p1 = psB.tile([64, nk], F32, tag="B0")
                nc.tensor.matmul(p1, lhsT=w1t, rhs=xvec, start=True, stop=True)
                h1 = gsb.tile([64, nk], F32, tag="m_h1")
                nc.scalar.activation(h1, p1, AF.Gelu, bias=b1)
                p2 = psB.tile([64, nk], F32, tag="B1")
                nc.tensor.matmul(p2, lhsT=w2t, rhs=h1, start=True, stop=True)
                h2 = gsb.tile([64, nk], F32, tag="m_h2")
                nc.scalar.activation(h2, p2, AF.Gelu, bias=b2)
                p3 = psB.tile([8, nk], F32, tag="B0")
                nc.tensor.matmul(p3, lhsT=w3t, rhs=h2, start=True, stop=True)
                at = gsb.tile([8, nk], F32, tag=f"m_at{role}")
                nc.scalar.activation(at, p3, AF.Identity, bias=b3)
                return at

            ghh = consts.tile([128, 2, 128], F32, tag="ghh")
            ghw = consts.tile([128, 2, 128], F32, tag="ghw")
            gp = {}
            gtags = {("w", 0): "A0", ("w", 1): "A1", ("h", 0): "B0", ("h", 1): "B1"}
            for (am, bm, nmk) in (("aw", "bc2", "w"), ("ah", "bc1", "h")):
                at = mlp_head(am, omega_t, 129, "a")
                bt = mlp_head(bm, lam_t, 128, "b")
                pool = psA if nmk == "w" else psB
                g0 = pool.tile([128, 128], F32, tag=gtags[(nmk, 0)])
                nc.tensor.matmul(g0, lhsT=at[:, 0:128], rhs=bt, start=True, stop=True)
                gn = pool.tile([1, 128], F32, tag=gtags[(nmk, 1)])
                nc.tensor.matmul(gn, lhsT=at[:, 128:129], rhs=bt, start=True, stop=True)
                gp[(nmk, 0)] = g0
                gp[(nmk, 1)] = gn
            # softplus(z) = relu(z) + log1p(exp(-|z|)), stage-batched across all 4
            keys = list(gp.keys())
            sp = {}
            for i, key in enumerate(keys):
                npart = 128 if key[1] == 0 else 1
                na = gsb.tile([128, 128], F32, tag=f"sp_na{i}")
                nc.scalar.activation(na[:npart, :], gp[key], AF.Abs)
                sp[key] = na
            for i, key in enumerate(keys):
                npart = 128 if key[1] == 0 else 1
                ex = gsb.tile([128, 128], F32, tag=f"sp_ex{i}")
                nc.scalar.activation(ex[:npart, :], sp[key][:npart, :], AF.Exp, scale=-1.0)
                nc.vector.tensor_scalar_add(ex[:npart, :], ex[:npart, :], 1.0)
                sp[key] = ex
            for key in keys:
                npart = 128 if key[1] == 0 else 1
                nc.scalar.activation(sp[key][:npart, :], sp[key][:npart, :], AF.Ln)
            for i, key in enumerate(keys):
                npart = 128 if key[1] == 0 else 1
                re = gsb.tile([128, 128], F32, tag=f"sp_na{i}")
                nc.scalar.activation(re[:npart, :], gp[key], AF.Relu)
                gh = ghh if key[0] == "h" else ghw
                if key[1] == 0:
                    nc.vector.tensor_add(gh[:, 0, :], sp[key][:128, :], re[:128, :])
                else:
                    # rows 128+j of ghat equal G[j]: copy the aligned block first,
                    # then overwrite row 0 with the Nyquist G[128].
                    nc.vector.tensor_copy(gh[:, 1, :], gh[:, 0, :])
                    nc.vector.tensor_add(gh[0:1, 1, :], sp[key][0:1, :], re[0:1, :])
            for gh in (ghh, ghw):
                nc.vector.tensor_scalar_mul(gh[:, :, :], gh[:, :, :], float(8.0 ** -0.5))

            # ---------------- BN prep ----------------
            bninv = consts.tile([128, 2], F32, tag="bninv")
            nc.vector.tensor_scalar_add(bninv, vec_t["bnv"], 1e-5)
            nc.scalar.activation(bninv, bninv, AF.Sqrt)
            nc.vector.reciprocal(bninv, bninv)
            nc.vector.tensor_tensor(out=bninv, in0=vec_t["bng"], in1=bninv, op=ALU.mult)
            bnbeff = consts.tile([128, 2], F32, tag="bnbeff")
            nc.vector.tensor_tensor(out=bnbeff, in0=vec_t["bnm"], in1=bninv, op=ALU.mult)
            nc.vector.tensor_tensor(out=bnbeff, in0=vec_t["bnb"], in1=bnbeff, op=ALU.subtract)

            u_t = ubuf.tile([128, 2, 128, HQ], BF16, tag="u")    # [k, kt, c, hq|w]
            usum = consts.tile([128, 2, 128], F32, tag="usum")   # [k, kt, c]

            def fwd_quarter(x_dram, idx, gate):
                """Load 16c-chunked segments, forward-transform, gate into u_t."""
                for cs in range(0, 128, 16):
                    xt = []
                    for t2 in (0, 1):
                        t = xin.tile([128, 16, HQ], BF16, tag=f"x{t2}")
                        nc.sync.dma_start(out=t, in_=x_dram[idx, t2, :, cs:cs + 16, :])
                        xt.append(t)
                    for kt in (0, 1):
                        pk = psA.tile([128, 16, HQ], F32, tag=f"A{kt}", name=f"fq{kt}")
                        for ch in (0, 8):
                            for t2 in (0, 1):
                                nc.tensor.matmul(pk[:, ch:ch + 8, :],
                                                 lhsT=tfwd_t[:, t2, kt * 128:(kt + 1) * 128],
                                                 rhs=xt[t2][:, ch:ch + 8, :],
                                                 start=(t2 == 0), stop=(t2 == 1))
                        nc.vector.tensor_tensor(
                            out=u_t[:, kt, cs:cs + 16, :],
                            in0=pk,
                            in1=gate[:, kt, cs:cs + 16].unsqueeze(2)
                                .broadcast_to([128, 16, HQ]),
                            op=ALU.mult)

            # =========== WC branch in h-quarters ===========
            y_w = [ybuf.tile([128, HQ, 128], BF16, tag=f"yw{q}", name=f"yw{q}")
                   for q in range(4)]
            for q in range(4):
                fwd_quarter(xw_d, q, ghw)
                # inverse: y_w[c, hq, w]; 8 h per PSUM group, one drain each
                for hb in range(0, HQ, 8):
                    pk = psB.tile([128, 8, 128], F32, tag=f"B{(hb // 8) % 2}",
                                  name=f"wi{(hb // 8) % 2}")
                    for i in range(8):
                        for kt in (0, 1):
                            nc.tensor.matmul(pk[:, i, :], lhsT=u_t[:, kt, :, hb + i],
                                             rhs=tinvw_t[:, kt, :],
                                             start=(kt == 0), stop=(kt == 1))
                    nc.scalar.activation(y_w[q][:, hb:hb + 8, :], pk, AF.Copy)
                # pooled-mean accumulation on the now-dead u: 2-level gpsimd
                # tree in place, then a small DVE reduce
                nc.gpsimd.tensor_add(u_t[:, :, :, 0:32], u_t[:, :, :, 0:32],
                                     u_t[:, :, :, 32:64])
                nc.gpsimd.tensor_add(u_t[:, :, :, 0:16], u_t[:, :, :, 0:16],
                                     u_t[:, :, :, 16:32])
                if q == 0:
                    nc.vector.tensor_reduce(out=usum, in_=u_t[:, :, :, 0:16],
                                            axis=AXX, op=ALU.add)
                else:
                    ur = gsb.tile([128, 2, 128], F32, tag="ur")
                    nc.vector.tensor_reduce(out=ur, in_=u_t[:, :, :, 0:16],
                                            axis=AXX, op=ALU.add)
                    nc.vector.tensor_add(usum, usum, ur)
                if q == 3:
                    # pool_w[c] = sum_k sigw[k] * usum[k, c]  -> AllReduce #1
                    pw_ps = psB.tile([128, 1], F32, tag="B0")
                    for kt in (0, 1):
                        nc.tensor.matmul(pw_ps, lhsT=usum[:, kt, :],
                                         rhs=sigw_t[:, kt:kt + 1],
                                         start=(kt == 0), stop=(kt == 1))
                    poolw_sb = gsb.tile([128, 1], F32, tag="poolw")
                    nc.vector.tensor_copy(poolw_sb, pw_ps)
                    nc.gpsimd.dma_start(out=arw_in[:], in_=poolw_sb)
                    nc.gpsimd.collective_compute(
                        "AllReduce", ALU.add,
                        replica_groups=[[0, 1], [2, 3], [4, 5], [6, 7]],
                        ins=[arw_in[:]], outs=[arw_out[:]])

            # =========== HC branch in w-halves ===========
            y_h = ybuf.tile([128, 256, 128], BF16, tag="yh")
            phacc = gsb.tile([1, 128], F32, tag="phacc")
            for ws in (0, 1):
                fwd_quarter(xh_d, ws, ghh)
                # inverse: y_h[c, h, w]; 4 w columns per PSUM group, one
                # (strided) drain each, alternating engines
                for wg in range(0, HQ, 4):
                    pk = psB.tile([128, 4, 256], F32, tag=f"B{(wg // 4) % 2}",
                                  name=f"hi{(wg // 4) % 2}")
                    for i in range(4):
                        for kt in (0, 1):
                            nc.tensor.matmul(pk[:, i, :], lhsT=u_t[:, kt, :, wg + i],
                                             rhs=tinv_t[:, kt, :],
                                             start=(kt == 0), stop=(kt == 1))
                    wl = ws * HQ + wg
                    src = pk.rearrange("c w h -> c h w")
                    if (wg // 4) % 2 == 0:
                        nc.vector.tensor_copy(y_h[:, :, wl:wl + 4], src)
                    else:
                        nc.scalar.activation(y_h[:, :, wl:wl + 4], src, AF.Copy)
                # DC-row trick: sum_{h,w} y_h = sqrt(N) * sum_w u[0, c, w]
                nc.gpsimd.tensor_add(u_t[0:1, 0, :, 0:32], u_t[0:1, 0, :, 0:32],
                                     u_t[0:1, 0, :, 32:64])
                phr = gsb.tile([1, 128], F32, tag=f"phrow{ws}", name=f"phrow{ws}")
                nc.vector.tensor_reduce(out=phr, in_=u_t[0:1, 0, :, 0:32],
                                        axis=AXX, op=ALU.add)
                if ws == 0:
                    nc.vector.tensor_copy(phacc, phr)
                else:
                    nc.vector.tensor_add(phacc, phacc, phr)
                    ph_ps = psB.tile([128, 1], F32, tag="B0")
                    nc.tensor.matmul(ph_ps, lhsT=phacc, rhs=one16_t,
                                     start=True, stop=True)
                    poolh_sb = gsb.tile([128, 1], F32, tag="poolh")
                    nc.vector.tensor_copy(poolh_sb, ph_ps)
                    nc.gpsimd.dma_start(out=arh_in[:], in_=poolh_sb)
                    nc.gpsimd.collective_compute(
                        "AllReduce", ALU.add,
                        replica_groups=[[0, 1], [2, 3], [4, 5], [6, 7]],
                        ins=[arh_in[:]], outs=[arh_out[:]])

            # ---------------- channel attention -> folded conv weights --------
            wsc = consts.tile([128, 2, 256], BF16, tag="wsc")
            p_sb = []
            for ct, aro in ((0, arh_out), (1, arw_out)):
                pt = gsb.tile([128, 1], F32, tag=f"p_ar{ct}")
                nc.gpsimd.dma_start(out=pt, in_=aro[:])
                p_sb.append(pt)
            q_sb = []
            for ot in (0, 1):
                q_ps = psB.tile([128, 1], F32, tag=f"B{ot}")
                for ct in (0, 1):
                    nc.tensor.matmul(q_ps, lhsT=caw1t_t[:, ct, ot * 128:(ot + 1) * 128],
                                     rhs=p_sb[ct], start=(ct == 0), stop=(ct == 1))
                qt = gsb.tile([128, 1], F32, tag=f"q{ot}")
                nc.scalar.activation(qt, q_ps, AF.Gelu, bias=vec_t["cab1"][:, ot:ot + 1])
                nc.gpsimd.tensor_tensor(out=qt, in0=qt, in1=vec_t["dwc"][:, ot:ot + 1],
                                        op=ALU.mult)
                q_sb.append(qt)
            for ot in (0, 1):
                s_t = gsb.tile([128, 1], F32, tag=f"s{ot}")
                nc.scalar.activation(s_t, q_sb[ot], AF.Sigmoid,
                                     bias=vec_t["dwb"][:, ot:ot + 1])
                nc.gpsimd.tensor_scalar_mul(wsc[:, ot, :], lcwt_t[:, ot, :], s_t)

            # ---------------- conv 1x1 + BN + GELU + residual + store ---------
            for hc in range(0, 256, 8):
                ywq = y_w[hc // HQ]
                hof = hc % HQ
                for ot in (0, 1):
                    xrt = stg.tile([128, 8, 128], BF16, tag=f"xr{ot}")
                    dq = nc.scalar if ot == 0 else nc.sync
                    dq.dma_start(out=xrt,
                                 in_=xres_d[ot * 128:(ot + 1) * 128, hc:hc + 8, :])
                    ostg = stg.tile([128, 8, 128], BF16, tag=f"g{ot}")
                    po = psA.tile([128, 8, 128], F32, tag=f"A{ot}", name=f"cv{ot}")
                    for sl in (0, 4):
                        nc.tensor.matmul(po[:, sl:sl + 4, :],
                                         lhsT=wsc[:, 0, ot * 128:(ot + 1) * 128],
                                         rhs=y_h[:, hc + sl:hc + sl + 4, :],
                                         start=True, stop=False)
                        nc.tensor.matmul(po[:, sl:sl + 4, :],
                                         lhsT=wsc[:, 1, ot * 128:(ot + 1) * 128],
                                         rhs=ywq[:, hof + sl:hof + sl + 4, :],
                                         start=False, stop=True)
                    nc.scalar.activation(ostg, po, AF.Gelu,
                                         bias=bnbeff[:, ot:ot + 1],
                                         scale=bninv[:, ot:ot + 1])
                    if ot == 0:
                        nc.vector.tensor_add(ostg, ostg, xrt)
                    else:
                        nc.gpsimd.tensor_add(ostg, ostg, xrt)
                    nc.gpsimd.dma_start(out=out_d[ot * 128:(ot + 1) * 128, hc:hc + 8, :],
                                        in_=ostg)

    nc.compile()
    return nc


_NC_CACHE = None


def _get_nc():
    global _NC_CACHE
    if _NC_CACHE is None:
        _NC_CACHE = _build()
    return _NC_CACHE


def _host_consts(inputs, core):
    """Per-core constant inputs (everything except the x shards)."""
    s = core % 2
    wlo = WS * s
    T = _dft_basis()
    d = {}
    d["tfwd"] = _part_major(np.ascontiguousarray(T.T)).astype(_BF16_NP)
    d["tinv"] = _part_major(T).astype(_BF16_NP)
    d["tinvw"] = _part_major(np.ascontiguousarray(T[:, wlo:wlo + WS])).astype(_BF16_NP)
    d["sigw"] = _part_major(T[:, wlo:wlo + WS].sum(axis=1)).astype(np.float32)
    d["omega"] = (np.arange(129, dtype=np.float32) / 128.0 - 1.0).reshape(1, 129)
    d["lam"] = np.linspace(-1.0, 1.0, 128, dtype=np.float32).reshape(1, 128)
    for m in _MLPS:
        d[f"{m}_w1t"] = np.ascontiguousarray(inputs[f"{m}_w1"].T).astype(np.float32)
        d[f"{m}_b1v"] = inputs[f"{m}_b1"].reshape(64, 1).astype(np.float32)
        d[f"{m}_w2t"] = np.ascontiguousarray(inputs[f"{m}_w2"].T).astype(np.float32)
        d[f"{m}_b2v"] = inputs[f"{m}_b2"].reshape(64, 1).astype(np.float32)
        d[f"{m}_w3t"] = np.ascontiguousarray(inputs[f"{m}_w3"].T).astype(np.float32)
        d[f"{m}_b3v"] = inputs[f"{m}_b3"].reshape(8, 1).astype(np.float32)
    d["caw1t"] = _part_major(np.ascontiguousarray(inputs["ca_w1"].T) / 65536.0).astype(np.float32)
    d["cab1"] = _part_major(inputs["ca_b1"]).astype(np.float32)
    d["dwc"] = _part_major(np.ascontiguousarray(inputs["ca_dw"][:, 1, 1])).astype(np.float32)
    d["dwb"] = _part_major(inputs["ca_db"]).astype(np.float32)
    d["lcwt"] = _part_major(np.ascontiguousarray(inputs["lc_w"].T)).astype(_BF16_NP)
    d["bng"] = _part_major(inputs["bn_g"]).astype(np.float32)
    d["bnb"] = _part_major(inputs["bn_b"]).astype(np.float32)
    d["bnm"] = _part_major(inputs["bn_m"]).astype(np.float32)
    d["bnv"] = _part_major(inputs["bn_v"]).astype(np.float32)
    return d


def kernel(**inputs):
    x = np.asarray(inputs["x"], np.float32)
    nc = _get_nc()

    in_maps = []
    for core in range(NCORES):
        b, s = core // 2, core % 2
        wlo = WS * s
        m = _host_consts(inputs, core)
        # xh: [2ws, 2ht, 128h, 128c, 64w]
        xhq = x[b, :C2, :, wlo:wlo + WS]             # (128c, 256h, 128w)
        m["xh"] = np.ascontiguousarray(
            xhq.reshape(C2, 2, 128, 2, HQ).transpose(3, 1, 2, 0, 4)).astype(_BF16_NP)
        # xw: [4q, 2wt, 128w, 128c, 64hq]
        xwq = x[b, C2:, :, :].reshape(C2, 4, HQ, N)
        m["xw"] = np.ascontiguousarray(
            xwq.transpose(1, 3, 0, 2).reshape(4, 2, 128, C2, HQ)).astype(_BF16_NP)
        m["xres"] = np.ascontiguousarray(x[b, :, :, wlo:wlo + WS]).astype(_BF16_NP)
        in_maps.append(m)

    trace = os.environ.get("BASS_KERNEL_TRACE", "0") == "1"
    res = bass_utils.run_bass_kernel_spmd(
        nc, in_maps, core_ids=list(range(NCORES)),
        trace=trace, trace_cores=list(range(NCORES)) if trace else None,
        stitch_traces=False)
    if trace and res.exec_time_ns is not None:
        print(f"HW exec time: {res.exec_time_ns} ns")
        print(f"   mean exec time: {res.mean_exec_time_ns} ns  "
              f"(slowest core {res.max_exec_time_core_id})")
        if res.instructions_and_trace is not None:
            print("   trace:", res.instructions_and_trace[1])

    out = np.empty((B, 2 * C2, N, N), np.float32)
    for core in range(NCORES):
        b, s = core // 2, core % 2
        wlo = WS * s
        out[b, :, :, wlo:wlo + WS] = res.results[core]["out"].astype(np.float32)
    return out


# revision 28
# speedup vs baseline: 1.5906x; 1.1535x over previous
"""Trainium2 Bass kernel for the spectral-gating network (nn_DAPSO).

Model (B=4, C=256, H=W=256):
  - channels 0:128   : y_h = irfft(Gh * rfft(x, axis=H))   (per-channel gate)
  - channels 128:256 : y_w = irfft(Gw * rfft(x, axis=W))
  - gates Gh/Gw from tiny MLPs (computed on device)
  - channel attention: s = sigmoid(dw(gelu(W1 @ mean_hw(y) + b)))  -> y *= s
  - y2 = gelu(BN(lc_w @ y));  out = x + y2

irfft(G*rfft(x)) along an axis of length N equals T^T diag(ghat) T x with T
the orthonormal real DFT basis, so both branches are dense TensorE matmuls.

Sharding: 8 cores = 4 batches x 2 w-halves; only cross-core traffic is two
1KB AllReduces of pooled channel means.

v4: all-SBUF dataflow (no DRAM scratch), AR-latency-free phase order
(WC -> HC -> conv), pooled means read off the gated spectrum u
(sum_h u for pool_w via gpsimd tree on the dead u buffer; DC row for
pool_h), data-stationary inverse transforms landing channel-major, and
big-op engine balance: 2-bank PSUM tiles, 16c forward chunks, 8h/4w
inverse drain groups, conv at 8h, residual add split vector/gpsimd.
Per-core HBM traffic: ~41MB read + 16MB written.
"""
import sys
import os

sys.path.insert(0, "/opt/trn_rl_repo")

import numpy as np
import ml_dtypes

import concourse.bacc as bacc
import concourse.mybir as mybir
import concourse.tile as tile
from concourse import bass_utils

F32 = mybir.dt.float32
BF16 = mybir.dt.bfloat16
AF = mybir.ActivationFunctionType
ALU = mybir.AluOpType
AXX = mybir.AxisListType.X

N = 256          # H = W
C2 = 128         # channels per branch
B = 4
NCORES = 8
WS = 128         # per-core w-slice width
HQ = 64          # WC h-quarter / HC w-half-of-half size

_BF16_NP = ml_dtypes.bfloat16


def _dft_basis():
    """Orthonormal real DFT basis T (N, N): y = T^T diag(ghat) T x == irfft(G*rfft(x))."""
    n = np.arange(N)
    k = np.arange(1, N // 2)
    T = np.zeros((N, N), np.float64)
    T[0, :] = 1.0 / np.sqrt(N)
    T[1:N // 2, :] = np.sqrt(2.0 / N) * np.cos(2 * np.pi * k[:, None] * n[None, :] / N)
    T[N // 2, :] = (1.0 / np.sqrt(N)) * ((-1.0) ** n)
    T[N // 2 + 1:, :] = np.sqrt(2.0 / N) * np.sin(2 * np.pi * k[:, None] * n[None, :] / N)
    return T.astype(np.float32)


def _part_major(a):
    """(256, ...) -> (128, 2, ...) partition-major layout."""
    a = np.asarray(a)
    return np.ascontiguousarray(a.reshape(2, 128, *a.shape[1:]).transpose(
        (1, 0) + tuple(range(2, a.ndim + 1))))


_MLPS = ("ah", "bc1", "aw", "bc2")


def _build():
    nc = bacc.Bacc("TRN2", target_bir_lowering=False, num_devices=NCORES)

    # ---------------- I/O declarations ----------------
    # xh: [2ws, 2ht, 128h, 128c, 64w]   (HC input, h on partitions)
    xh_d = nc.dram_tensor("xh", [2, 2, 128, 128, HQ], BF16, kind="ExternalInput")
    # xw: [4q, 2wt, 128w, 128c, 64h]    (WC input, w on partitions)
    xw_d = nc.dram_tensor("xw", [4, 2, 128, 128, HQ], BF16, kind="ExternalInput")
    xres_d = nc.dram_tensor("xres", [256, 256, 128], BF16, kind="ExternalInput")
    tfwd_d = nc.dram_tensor("tfwd", [128, 2, 256], BF16, kind="ExternalInput")
    tinv_d = nc.dram_tensor("tinv", [128, 2, 256], BF16, kind="ExternalInput")
    tinvw_d = nc.dram_tensor("tinvw", [128, 2, 128], BF16, kind="ExternalInput")
    # dcw[0, q] = sqrt(N) if WC h-quarter q belongs to this core's pool
    # partial (pairs split the pooled sum by h-half instead of w-half --
    # summing Tinv columns over ALL w kills every non-DC row), else 0.
    dcw_d = nc.dram_tensor("dcw", [1, 4], F32, kind="ExternalInput")
    omega_d = nc.dram_tensor("omega", [1, 129], F32, kind="ExternalInput")
    lam_d = nc.dram_tensor("lam", [1, 128], F32, kind="ExternalInput")
    mlp_d = {}
    for m in _MLPS:
        mlp_d[m] = dict(
            w1t=nc.dram_tensor(f"{m}_w1t", [1, 64], F32, kind="ExternalInput"),
            b1=nc.dram_tensor(f"{m}_b1v", [64, 1], F32, kind="ExternalInput"),
            w2t=nc.dram_tensor(f"{m}_w2t", [64, 64], F32, kind="ExternalInput"),
            b2=nc.dram_tensor(f"{m}_b2v", [64, 1], F32, kind="ExternalInput"),
            w3t=nc.dram_tensor(f"{m}_w3t", [64, 8], F32, kind="ExternalInput"),
            b3=nc.dram_tensor(f"{m}_b3v", [8, 1], F32, kind="ExternalInput"),
        )
    caw1t_d = nc.dram_tensor("caw1t", [128, 2, 256], F32, kind="ExternalInput")
    cab1_d = nc.dram_tensor("cab1", [128, 2], F32, kind="ExternalInput")
    dwc_d = nc.dram_tensor("dwc", [128, 2], F32, kind="ExternalInput")
    dwb_d = nc.dram_tensor("dwb", [128, 2], F32, kind="ExternalInput")
    lcwt_d = nc.dram_tensor("lcwt", [128, 2, 256], BF16, kind="ExternalInput")
    bng_d = nc.dram_tensor("bng", [128, 2], F32, kind="ExternalInput")
    bnb_d = nc.dram_tensor("bnb", [128, 2], F32, kind="ExternalInput")
    bnm_d = nc.dram_tensor("bnm", [128, 2], F32, kind="ExternalInput")
    bnv_d = nc.dram_tensor("bnv", [128, 2], F32, kind="ExternalInput")

    out_d = nc.dram_tensor("out", [256, 256, 128], BF16, kind="ExternalOutput")

    arh_in = nc.dram_tensor("arh_in", [128, 1], F32)
    arh_out = nc.dram_tensor("arh_out", [128, 1], F32)
    arw_in = nc.dram_tensor("arw_in", [128, 1], F32)
    arw_out = nc.dram_tensor("arw_out", [128, 1], F32)

    with tile.TileContext(nc) as tc:
        with tc.tile_pool(name="consts", bufs=1) as consts, \
             tc.tile_pool(name="xin", bufs=2) as xin, \
             tc.tile_pool(name="ubuf", bufs=1) as ubuf, \
             tc.tile_pool(name="ybuf", bufs=1) as ybuf, \
             tc.tile_pool(name="stg", bufs=2) as stg, \
             tc.tile_pool(name="gsb", bufs=1) as gsb, \
             tc.tile_pool(name="psA", bufs=1, space="PSUM") as psA, \
             tc.tile_pool(name="psB", bufs=1, space="PSUM") as psB:

            # ---------------- const loads ----------------
            tfwd_t = consts.tile([128, 2, 256], BF16, tag="tfwd")
            nc.sync.dma_start(out=tfwd_t, in_=tfwd_d[:])
            tinv_t = consts.tile([128, 2, 256], BF16, tag="tinv")
            nc.sync.dma_start(out=tinv_t, in_=tinv_d[:])
            tinvw_t = consts.tile([128, 2, 128], BF16, tag="tinvw")
            nc.sync.dma_start(out=tinvw_t, in_=tinvw_d[:])
            dcw_t = consts.tile([1, 4], F32, tag="dcw")
            nc.sync.dma_start(out=dcw_t, in_=dcw_d[:])
            caw1t_t = consts.tile([128, 2, 256], F32, tag="caw1t")
            nc.sync.dma_start(out=caw1t_t, in_=caw1t_d[:])
            lcwt_t = consts.tile([128, 2, 256], BF16, tag="lcwt")
            nc.sync.dma_start(out=lcwt_t, in_=lcwt_d[:])
            vec_t = {}
            for nm, d in (("cab1", cab1_d), ("dwc", dwc_d), ("dwb", dwb_d),
                          ("bng", bng_d), ("bnb", bnb_d), ("bnm", bnm_d), ("bnv", bnv_d)):
                vt = consts.tile([128, 2], F32, tag=f"v_{nm}")
                nc.sync.dma_start(out=vt, in_=d[:])
                vec_t[nm] = vt
            omega_t = consts.tile([1, 129], F32, tag="omega")
            nc.sync.dma_start(out=omega_t, in_=omega_d[:])
            lam_t = consts.tile([1, 128], F32, tag="lam")
            nc.sync.dma_start(out=lam_t, in_=lam_d[:])
            one16_t = consts.tile([1, 1], F32, tag="one16")
            nc.vector.memset(one16_t, float(np.sqrt(N)))

            # ---------------- gate MLPs (tiny), table-batched ----------------
            def mlp_head(m, xvec, nk, role):
                d = mlp_d[m]
                w1t = gsb.tile([1, 64], F32, tag="m_w1")
                nc.sync.dma_start(out=w1t, in_=d["w1t"][:])
                b1 = gsb.tile([64, 1], F32, tag="m_b1")
                nc.sync.dma_start(out=b1, in_=d["b1"][:])
                w2t = gsb.tile([64, 64], F32, tag="m_w2")
                nc.sync.dma_start(out=w2t, in_=d["w2t"][:])
                b2 = gsb.tile([64, 1], F32, tag="m_b2")
                nc.sync.dma_start(out=b2, in_=d["b2"][:])
                w3t = gsb.tile([64, 8], F32, tag="m_w3")
                nc.sync.dma_start(out=w3t, in_=d["w3t"][:])
                b3 = gsb.tile([8, 1], F32, tag="m_b3")
                nc.sync.dma_start(out=b3, in_=d["b3"][:])

                p1 = psB.tile([64, nk], F32, tag="B0")
                nc.tensor.matmul(p1, lhsT=w1t, rhs=xvec, start=True, stop=True)
                h1 = gsb.tile([64, nk], F32, tag="m_h1")
                nc.scalar.activation(h1, p1, AF.Gelu, bias=b1)
                p2 = psB.tile([64, nk], F32, tag="B1")
                nc.tensor.matmul(p2, lhsT=w2t, rhs=h1, start=True, stop=True)
                h2 = gsb.tile([64, nk], F32, tag="m_h2")
                nc.scalar.activation(h2, p2, AF.Gelu, bias=b2)
                p3 = psB.tile([8, nk], F32, tag="B0")
                nc.tensor.matmul(p3, lhsT=w3t, rhs=h2, start=True, stop=True)
                at = gsb.tile([8, nk], F32, tag=f"m_at{role}")
                nc.scalar.activation(at, p3, AF.Identity, bias=b3)
                return at

            ghh = consts.tile([128, 2, 128], F32, tag="ghh")
            ghw = consts.tile([128, 2, 128], F32, tag="ghw")
            gp = {}
            gtags = {("w", 0): "A0", ("w", 1): "A1", ("h", 0): "B0", ("h", 1): "B1"}
            for (am, bm, nmk) in (("aw", "bc2", "w"), ("ah", "bc1", "h")):
                at = mlp_head(am, omega_t, 129, "a")
                bt = mlp_head(bm, lam_t, 128, "b")
                pool = psA if nmk == "w" else psB
                g0 = pool.tile([128, 128], F32, tag=gtags[(nmk, 0)])
                nc.tensor.matmul(g0, lhsT=at[:, 0:128], rhs=bt, start=True, stop=True)
                gn = pool.tile([1, 128], F32, tag=gtags[(nmk, 1)])
                nc.tensor.matmul(gn, lhsT=at[:, 128:129], rhs=bt, start=True, stop=True)
                gp[(nmk, 0)] = g0
                gp[(nmk, 1)] = gn
            # softplus(z) = relu(z) + log1p(exp(-|z|)), stage-batched across all 4
            keys = list(gp.keys())
            sp = {}
            for i, key in enumerate(keys):
                npart = 128 if key[1] == 0 else 1
                na = gsb.tile([128, 128], F32, tag=f"sp_na{i}")
                nc.scalar.activation(na[:npart, :], gp[key], AF.Abs)
                sp[key] = na
            for i, key in enumerate(keys):
                npart = 128 if key[1] == 0 else 1
                ex = gsb.tile([128, 128], F32, tag=f"sp_ex{i}")
                nc.scalar.activation(ex[:npart, :], sp[key][:npart, :], AF.Exp, scale=-1.0)
                nc.vector.tensor_scalar_add(ex[:npart, :], ex[:npart, :], 1.0)
                sp[key] = ex
            for key in keys:
                npart = 128 if key[1] == 0 else 1
                nc.scalar.activation(sp[key][:npart, :], sp[key][:npart, :], AF.Ln)
            for i, key in enumerate(keys):
                npart = 128 if key[1] == 0 else 1
                re = gsb.tile([128, 128], F32, tag=f"sp_na{i}")
                nc.scalar.activation(re[:npart, :], gp[key], AF.Relu)
                gh = ghh if key[0] == "h" else ghw
                if key[1] == 0:
                    nc.vector.tensor_add(gh[:, 0, :], sp[key][:128, :], re[:128, :])
                else:
                    # rows 128+j of ghat equal G[j]: copy the aligned block first,
                    # then overwrite row 0 with the Nyquist G[128].
                    nc.vector.tensor_copy(gh[:, 1, :], gh[:, 0, :])
                    nc.vector.tensor_add(gh[0:1, 1, :], sp[key][0:1, :], re[0:1, :])
            for gh in (ghh, ghw):
                nc.vector.tensor_scalar_mul(gh[:, :, :], gh[:, :, :], float(8.0 ** -0.5))

            # ---------------- BN prep ----------------
            bninv = consts.tile([128, 2], F32, tag="bninv")
            nc.vector.tensor_scalar_add(bninv, vec_t["bnv"], 1e-5)
            nc.scalar.activation(bninv, bninv, AF.Sqrt)
            nc.vector.reciprocal(bninv, bninv)
            nc.vector.tensor_tensor(out=bninv, in0=vec_t["bng"], in1=bninv, op=ALU.mult)
            bnbeff = consts.tile([128, 2], F32, tag="bnbeff")
            nc.vector.tensor_tensor(out=bnbeff, in0=vec_t["bnm"], in1=bninv, op=ALU.mult)
            nc.vector.tensor_tensor(out=bnbeff, in0=vec_t["bnb"], in1=bnbeff, op=ALU.subtract)

            u_t = ubuf.tile([128, 2, 128, HQ], BF16, tag="u")    # [k, kt, c, hq|w]

            def fwd_quarter(x_dram, idx, gate):
                """Load 16c-chunked segments, forward-transform, gate into u_t."""
                for cs in range(0, 128, 16):
                    xt = []
                    for t2 in (0, 1):
                        t = xin.tile([128, 16, HQ], BF16, tag=f"x{t2}")
                        nc.sync.dma_start(out=t, in_=x_dram[idx, t2, :, cs:cs + 16, :])
                        xt.append(t)
                    for kt in (0, 1):
                        pk = psA.tile([128, 16, HQ], F32, tag=f"A{kt}", name=f"fq{kt}")
                        for ch in (0, 8):
                            for t2 in (0, 1):
                                nc.tensor.matmul(pk[:, ch:ch + 8, :],
                                                 lhsT=tfwd_t[:, t2, kt * 128:(kt + 1) * 128],
                                                 rhs=xt[t2][:, ch:ch + 8, :],
                                                 start=(t2 == 0), stop=(t2 == 1))
                        nc.vector.tensor_tensor(
                            out=u_t[:, kt, cs:cs + 16, :],
                            in0=pk,
                            in1=gate[:, kt, cs:cs + 16].unsqueeze(2)
                                .broadcast_to([128, 16, HQ]),
                            op=ALU.mult)

            # =========== WC branch in h-quarters ===========
            # pool_w partial = sum over this core's h-HALF of the DC row of u
            # (summing Tinv columns over all 256 w kills every k != 0 term);
            # the pair AllReduce then covers the full (h, w) extent.
            y_w = [ybuf.tile([128, HQ, 128], BF16, tag=f"yw{q}", name=f"yw{q}")
                   for q in range(4)]
            pwacc = gsb.tile([1, 128], F32, tag="pwacc")
            for q in range(4):
                fwd_quarter(xw_d, q, ghw)
                # DC-row pool partial (on live u, before the inverse)
                pwr = gsb.tile([1, 128], F32, tag="pwr")
                nc.vector.tensor_reduce(out=pwr, in_=u_t[0:1, 0, :, :],
                                        axis=AXX, op=ALU.add)
                nc.vector.tensor_scalar_mul(pwr, pwr, dcw_t[0:1, q:q + 1])
                if q == 0:
                    nc.vector.tensor_copy(pwacc, pwr)
                else:
                    nc.vector.tensor_add(pwacc, pwacc, pwr)
                if q == 3:
                    pw_ps = psB.tile([128, 1], F32, tag="B0")
                    nc.tensor.matmul(pw_ps, lhsT=pwacc, rhs=one16_t,
                                     start=True, stop=True)
                    poolw_sb = gsb.tile([128, 1], F32, tag="poolw")
                    nc.vector.tensor_copy(poolw_sb, pw_ps)
                    nc.gpsimd.dma_start(out=arw_in[:], in_=poolw_sb)
                    nc.gpsimd.collective_compute(
                        "AllReduce", ALU.add,
                        replica_groups=[[0, 1], [2, 3], [4, 5], [6, 7]],
                        ins=[arw_in[:]], outs=[arw_out[:]])
                # inverse: y_w[c, hq, w]; 8 h per PSUM group, one drain each
                for hb in range(0, HQ, 8):
                    pk = psB.tile([128, 8, 128], F32, tag=f"B{(hb // 8) % 2}",
                                  name=f"wi{(hb // 8) % 2}")
                    for i in range(8):
                        for kt in (0, 1):
                            nc.tensor.matmul(pk[:, i, :], lhsT=u_t[:, kt, :, hb + i],
                                             rhs=tinvw_t[:, kt, :],
                                             start=(kt == 0), stop=(kt == 1))
                    nc.scalar.activation(y_w[q][:, hb:hb + 8, :], pk, AF.Copy)

            # =========== HC branch in w-halves ===========
            y_h = ybuf.tile([128, 256, 128], BF16, tag="yh")
            phacc = gsb.tile([1, 128], F32, tag="phacc")
            for ws in (0, 1):
                fwd_quarter(xh_d, ws, ghh)
                # DC-row trick: sum_{h,w} y_h = sqrt(N) * sum_w u[0, c, w]
                # (fired before the inverse so the AR hides behind it)
                phr = gsb.tile([1, 128], F32, tag=f"phrow{ws}", name=f"phrow{ws}")
                nc.vector.tensor_reduce(out=phr, in_=u_t[0:1, 0, :, :],
                                        axis=AXX, op=ALU.add)
                if ws == 0:
                    nc.vector.tensor_copy(phacc, phr)
                else:
                    nc.vector.tensor_add(phacc, phacc, phr)
                # inverse: y_h[c, h, w]; 4 w columns per PSUM group, one
                # (strided) drain each, alternating engines.  The pool
                # matmul + AR-h are injected mid-loop (ws 1) so the AR
                # round-trip hides behind the inverse's tail.
                for wg in range(0, HQ, 4):
                    if ws == 1 and wg == 24:
                        ph_ps = psB.tile([128, 1], F32, tag="B0")
                        nc.tensor.matmul(ph_ps, lhsT=phacc, rhs=one16_t,
                                         start=True, stop=True)
                        poolh_sb = gsb.tile([128, 1], F32, tag="poolh")
                        nc.vector.tensor_copy(poolh_sb, ph_ps)
                        nc.gpsimd.dma_start(out=arh_in[:], in_=poolh_sb)
                        nc.gpsimd.collective_compute(
                            "AllReduce", ALU.add,
                            replica_groups=[[0, 1], [2, 3], [4, 5], [6, 7]],
                            ins=[arh_in[:]], outs=[arh_out[:]])
                    pk = psB.tile([128, 4, 256], F32, tag=f"B{(wg // 4) % 2}",
                                  name=f"hi{(wg // 4) % 2}")
                    for i in range(4):
                        for kt in (0, 1):
                            nc.tensor.matmul(pk[:, i, :], lhsT=u_t[:, kt, :, wg + i],
                                             rhs=tinv_t[:, kt, :],
                                             start=(kt == 0), stop=(kt == 1))
                    wl = ws * HQ + wg
                    src = pk.rearrange("c w h -> c h w")
                    if (wg // 4) % 2 == 0:
                        nc.vector.tensor_copy(y_h[:, :, wl:wl + 4], src)
                    else:
                        nc.scalar.activation(y_h[:, :, wl:wl + 4], src, AF.Copy)

            # ---------------- channel attention -> folded conv weights --------
            wsc = consts.tile([128, 2, 256], BF16, tag="wsc")
            p_sb = []
            for ct, aro in ((0, arh_out), (1, arw_out)):
                pt = gsb.tile([128, 1], F32, tag=f"p_ar{ct}")
                nc.gpsimd.dma_start(out=pt, in_=aro[:])
                p_sb.append(pt)
            q_sb = []
            for ot in (0, 1):
                q_ps = psB.tile([128, 1], F32, tag=f"B{ot}")
                for ct in (0, 1):
                    nc.tensor.matmul(q_ps, lhsT=caw1t_t[:, ct, ot * 128:(ot + 1) * 128],
                                     rhs=p_sb[ct], start=(ct == 0), stop=(ct == 1))
                qt = gsb.tile([128, 1], F32, tag=f"q{ot}")
                nc.scalar.activation(qt, q_ps, AF.Gelu, bias=vec_t["cab1"][:, ot:ot + 1])
                nc.gpsimd.tensor_tensor(out=qt, in0=qt, in1=vec_t["dwc"][:, ot:ot + 1],
                                        op=ALU.mult)
                q_sb.append(qt)
            for ot in (0, 1):
                s_t = gsb.tile([128, 1], F32, tag=f"s{ot}")
                nc.scalar.activation(s_t, q_sb[ot], AF.Sigmoid,
                                     bias=vec_t["dwb"][:, ot:ot + 1])
                nc.gpsimd.tensor_scalar_mul(wsc[:, ot, :], lcwt_t[:, ot, :], s_t)

            # ---------------- conv 1x1 + BN + GELU + residual + store ---------
            for hc in range(0, 256, 8):
                ywq = y_w[hc // HQ]
                hof = hc % HQ
                for ot in (0, 1):
                    xrt = stg.tile([128, 8, 128], BF16, tag=f"xr{ot}")
                    dq = nc.scalar if ot == 0 else nc.sync
                    dq.dma_start(out=xrt,
                                 in_=xres_d[ot * 128:(ot + 1) * 128, hc:hc + 8, :])
                    ostg = stg.tile([128, 8, 128], BF16, tag=f"g{ot}")
                    po = psA.tile([128, 8, 128], F32, tag=f"A{ot}", name=f"cv{ot}")
                    for sl in (0, 4):
                        nc.tensor.matmul(po[:, sl:sl + 4, :],
                                         lhsT=wsc[:, 0, ot * 128:(ot + 1) * 128],
                                         rhs=y_h[:, hc + sl:hc + sl + 4, :],
                                         start=True, stop=False)
                        nc.tensor.matmul(po[:, sl:sl + 4, :],
                                         lhsT=wsc[:, 1, ot * 128:(ot + 1) * 128],
                                         rhs=ywq[:, hof + sl:hof + sl + 4, :],
                                         start=False, stop=True)
                    nc.scalar.activation(ostg, po, AF.Gelu,
                                         bias=bnbeff[:, ot:ot + 1],
                                         scale=bninv[:, ot:ot + 1])
                    if ot == 0 or (hc // 8) % 2 == 0:
                        nc.vector.tensor_add(ostg, ostg, xrt)
                    else:
                        nc.gpsimd.tensor_add(ostg, ostg, xrt)
                    nc.gpsimd.dma_start(out=out_d[ot * 128:(ot + 1) * 128, hc:hc + 8, :],
                                        in_=ostg)

    nc.compile()
    return nc


_NC_CACHE = None


def _get_nc():
    global _NC_CACHE
    if _NC_CACHE is None:
        _NC_CACHE = _build()
    return _NC_CACHE


def _host_consts(inputs, core):
    """Per-core constant inputs (everything except the x shards)."""
    s = core % 2
    wlo = WS * s
    T = _dft_basis()
    d = {}
    d["tfwd"] = _part_major(np.ascontiguousarray(T.T)).astype(_BF16_NP)
    d["tinv"] = _part_major(T).astype(_BF16_NP)
    d["tinvw"] = _part_major(np.ascontiguousarray(T[:, wlo:wlo + WS])).astype(_BF16_NP)
    d["dcw"] = np.array([[1.0 if q // 2 == s else 0.0 for q in range(4)]],
                        np.float32)
    d["omega"] = (np.arange(129, dtype=np.float32) / 128.0 - 1.0).reshape(1, 129)
    d["lam"] = np.linspace(-1.0, 1.0, 128, dtype=np.float32).reshape(1, 128)
    for m in _MLPS:
        d[f"{m}_w1t"] = np.ascontiguousarray(inputs[f"{m}_w1"].T).astype(np.float32)
        d[f"{m}_b1v"] = inputs[f"{m}_b1"].reshape(64, 1).astype(np.float32)
        d[f"{m}_w2t"] = np.ascontiguousarray(inputs[f"{m}_w2"].T).astype(np.float32)
        d[f"{m}_b2v"] = inputs[f"{m}_b2"].reshape(64, 1).astype(np.float32)
        d[f"{m}_w3t"] = np.ascontiguousarray(inputs[f"{m}_w3"].T).astype(np.float32)
        d[f"{m}_b3v"] = inputs[f"{m}_b3"].reshape(8, 1).astype(np.float32)
    d["caw1t"] = _part_major(np.ascontiguousarray(inputs["ca_w1"].T) / 65536.0).astype(np.float32)
    d["cab1"] = _part_major(inputs["ca_b1"]).astype(np.float32)
    d["dwc"] = _part_major(np.ascontiguousarray(inputs["ca_dw"][:, 1, 1])).astype(np.float32)
    d["dwb"] = _part_major(inputs["ca_db"]).astype(np.float32)
    d["lcwt"] = _part_major(np.ascontiguousarray(inputs["lc_w"].T)).astype(_BF16_NP)
    d["bng"] = _part_major(inputs["bn_g"]).astype(np.float32)
    d["bnb"] = _part_major(inputs["bn_b"]).astype(np.float32)
    d["bnm"] = _part_major(inputs["bn_m"]).astype(np.float32)
    d["bnv"] = _part_major(inputs["bn_v"]).astype(np.float32)
    return d


def kernel(**inputs):
    x = np.asarray(inputs["x"], np.float32)
    nc = _get_nc()

    in_maps = []
    for core in range(NCORES):
        b, s = core // 2, core % 2
        wlo = WS * s
        m = _host_consts(inputs, core)
        # xh: [2ws, 2ht, 128h, 128c, 64w]
        xhq = x[b, :C2, :, wlo:wlo + WS]             # (128c, 256h, 128w)
        m["xh"] = np.ascontiguousarray(
            xhq.reshape(C2, 2, 128, 2, HQ).transpose(3, 1, 2, 0, 4)).astype(_BF16_NP)
        # xw: [4q, 2wt, 128w, 128c, 64hq]
        xwq = x[b, C2:, :, :].reshape(C2, 4, HQ, N)
        m["xw"] = np.ascontiguousarray(
            xwq.transpose(1, 3, 0, 2).reshape(4, 2, 128, C2, HQ)).astype(_BF16_NP)
        m["xres"] = np.ascontiguousarray(x[b, :, :, wlo:wlo + WS]).astype(_BF16_NP)
        in_maps.append(m)

    trace = os.environ.get("BASS_KERNEL_TRACE", "0") == "1"
    res = bass_utils.run_bass_kernel_spmd(
        nc, in_maps, core_ids=list(range(NCORES)),
        trace=trace, trace_cores=list(range(NCORES)) if trace else None,
        stitch_traces=False)
    if trace and res.exec_time_ns is not None:
        print(f"HW exec time: {res.exec_time_ns} ns")
        print(f"   mean exec time: {res.mean_exec_time_ns} ns  "
              f"(slowest core {res.max_exec_time_core_id})")
        if res.instructions_and_trace is not None:
            print("   trace:", res.instructions_and_trace[1])

    out = np.empty((B, 2 * C2, N, N), np.float32)
    for core in range(NCORES):
        b, s = core // 2, core % 2
        wlo = WS * s
        out[b, :, :, wlo:wlo + WS] = res.results[core]["out"].astype(np.float32)
    return out


# revision 36
# speedup vs baseline: 1.7058x; 1.0724x over previous
"""Trainium2 Bass kernel for the spectral-gating network (nn_DAPSO).

Model (B=4, C=256, H=W=256):
  - channels 0:128   : y_h = irfft(Gh * rfft(x, axis=H))   (per-channel gate)
  - channels 128:256 : y_w = irfft(Gw * rfft(x, axis=W))
  - gates Gh/Gw from tiny MLPs (computed on device)
  - channel attention: s = sigmoid(dw(gelu(W1 @ mean_hw(y) + b)))  -> y *= s
  - y2 = gelu(BN(lc_w @ y));  out = x + y2

irfft(G*rfft(x)) along an axis of length N equals T^T diag(ghat) T x with T
the orthonormal real DFT basis, so both branches are dense TensorE matmuls.

Sharding: 8 cores = 4 batches x 2 w-halves; only cross-core traffic is two
1KB AllReduces of pooled channel means.

v4: all-SBUF dataflow (no DRAM scratch), AR-latency-free phase order
(WC -> HC -> conv), pooled means read off the gated spectrum u
(sum_h u for pool_w via gpsimd tree on the dead u buffer; DC row for
pool_h), data-stationary inverse transforms landing channel-major, and
big-op engine balance: 2-bank PSUM tiles, 16c forward chunks, 8h/4w
inverse drain groups, conv at 8h, residual add split vector/gpsimd.
Per-core HBM traffic: ~41MB read + 16MB written.
"""
import sys
import os

sys.path.insert(0, "/opt/trn_rl_repo")

import numpy as np
import ml_dtypes

import concourse.bacc as bacc
import concourse.mybir as mybir
import concourse.tile as tile
from concourse import bass_utils

F32 = mybir.dt.float32
BF16 = mybir.dt.bfloat16
AF = mybir.ActivationFunctionType
ALU = mybir.AluOpType
AXX = mybir.AxisListType.X

N = 256          # H = W
C2 = 128         # channels per branch
B = 4
NCORES = 8
WS = 128         # per-core w-slice width
HQ = 64          # WC h-quarter / HC w-half-of-half size

_BF16_NP = ml_dtypes.bfloat16


def _dft_basis():
    """Orthonormal real DFT basis T (N, N): y = T^T diag(ghat) T x == irfft(G*rfft(x))."""
    n = np.arange(N)
    k = np.arange(1, N // 2)
    T = np.zeros((N, N), np.float64)
    T[0, :] = 1.0 / np.sqrt(N)
    T[1:N // 2, :] = np.sqrt(2.0 / N) * np.cos(2 * np.pi * k[:, None] * n[None, :] / N)
    T[N // 2, :] = (1.0 / np.sqrt(N)) * ((-1.0) ** n)
    T[N // 2 + 1:, :] = np.sqrt(2.0 / N) * np.sin(2 * np.pi * k[:, None] * n[None, :] / N)
    return T.astype(np.float32)


def _part_major(a):
    """(256, ...) -> (128, 2, ...) partition-major layout."""
    a = np.asarray(a)
    return np.ascontiguousarray(a.reshape(2, 128, *a.shape[1:]).transpose(
        (1, 0) + tuple(range(2, a.ndim + 1))))


_MLPS = ("ah", "bc1", "aw", "bc2")


def _build():
    nc = bacc.Bacc("TRN2", target_bir_lowering=False, num_devices=NCORES)

    # ---------------- I/O declarations ----------------
    # xh: [2ws, 2ht, 128h, 128c, 64w]   (HC input, h on partitions)
    xh_d = nc.dram_tensor("xh", [2, 2, 128, 128, HQ], BF16, kind="ExternalInput")
    # xw: [4q, 2wt, 128w, 128c, 64h]    (WC input, w on partitions)
    xw_d = nc.dram_tensor("xw", [4, 2, 128, 128, HQ], BF16, kind="ExternalInput")
    xres_d = nc.dram_tensor("xres", [256, 256, 128], BF16, kind="ExternalInput")
    tfwd_d = nc.dram_tensor("tfwd", [128, 2, 256], BF16, kind="ExternalInput")
    tinv_d = nc.dram_tensor("tinv", [128, 2, 256], BF16, kind="ExternalInput")
    tinvw_d = nc.dram_tensor("tinvw", [128, 2, 128], BF16, kind="ExternalInput")
    # mkw[c, q] = sqrt(N) if WC h-quarter q belongs to this core's pool
    # partial (pairs split the pooled sum by h-half instead of w-half --
    # summing Tinv columns over ALL w kills every non-DC row), else 0.
    mkw_d = nc.dram_tensor("mkw", [128, 4], F32, kind="ExternalInput")
    omega_d = nc.dram_tensor("omega", [1, 129], F32, kind="ExternalInput")
    lam_d = nc.dram_tensor("lam", [1, 128], F32, kind="ExternalInput")
    mlp_d = {}
    for m in _MLPS:
        mlp_d[m] = dict(
            w1t=nc.dram_tensor(f"{m}_w1t", [1, 64], F32, kind="ExternalInput"),
            b1=nc.dram_tensor(f"{m}_b1v", [64, 1], F32, kind="ExternalInput"),
            w2t=nc.dram_tensor(f"{m}_w2t", [64, 64], F32, kind="ExternalInput"),
            b2=nc.dram_tensor(f"{m}_b2v", [64, 1], F32, kind="ExternalInput"),
            w3t=nc.dram_tensor(f"{m}_w3t", [64, 8], F32, kind="ExternalInput"),
            b3=nc.dram_tensor(f"{m}_b3v", [8, 1], F32, kind="ExternalInput"),
        )
    caw1t_d = nc.dram_tensor("caw1t", [128, 2, 256], F32, kind="ExternalInput")
    cab1_d = nc.dram_tensor("cab1", [128, 2], F32, kind="ExternalInput")
    dwc_d = nc.dram_tensor("dwc", [128, 2], F32, kind="ExternalInput")
    dwb_d = nc.dram_tensor("dwb", [128, 2], F32, kind="ExternalInput")
    lcwt_d = nc.dram_tensor("lcwt", [128, 2, 256], BF16, kind="ExternalInput")
    bng_d = nc.dram_tensor("bng", [128, 2], F32, kind="ExternalInput")
    bnb_d = nc.dram_tensor("bnb", [128, 2], F32, kind="ExternalInput")
    bnm_d = nc.dram_tensor("bnm", [128, 2], F32, kind="ExternalInput")
    bnv_d = nc.dram_tensor("bnv", [128, 2], F32, kind="ExternalInput")

    out_d = nc.dram_tensor("out", [256, 256, 128], BF16, kind="ExternalOutput")

    arh_in = nc.dram_tensor("arh_in", [128, 1], F32)
    arh_out = nc.dram_tensor("arh_out", [128, 1], F32)
    arw_in = nc.dram_tensor("arw_in", [128, 1], F32)
    arw_out = nc.dram_tensor("arw_out", [128, 1], F32)

    with tile.TileContext(nc) as tc:
        with tc.tile_pool(name="consts", bufs=1) as consts, \
             tc.tile_pool(name="xin", bufs=2) as xin, \
             tc.tile_pool(name="ubuf", bufs=1) as ubuf, \
             tc.tile_pool(name="ybuf", bufs=1) as ybuf, \
             tc.tile_pool(name="stg", bufs=2) as stg, \
             tc.tile_pool(name="gsb", bufs=1) as gsb, \
             tc.tile_pool(name="psA", bufs=1, space="PSUM") as psA, \
             tc.tile_pool(name="psB", bufs=1, space="PSUM") as psB:

            # ---------------- const loads ----------------
            tfwd_t = consts.tile([128, 2, 256], BF16, tag="tfwd")
            nc.sync.dma_start(out=tfwd_t, in_=tfwd_d[:])
            tinv_t = consts.tile([128, 2, 256], BF16, tag="tinv")
            nc.sync.dma_start(out=tinv_t, in_=tinv_d[:])
            tinvw_t = consts.tile([128, 2, 128], BF16, tag="tinvw")
            nc.sync.dma_start(out=tinvw_t, in_=tinvw_d[:])
            mkw_t = consts.tile([128, 4], F32, tag="mkw")
            nc.sync.dma_start(out=mkw_t, in_=mkw_d[:])
            caw1t_t = consts.tile([128, 2, 256], F32, tag="caw1t")
            nc.sync.dma_start(out=caw1t_t, in_=caw1t_d[:])
            lcwt_t = consts.tile([128, 2, 256], BF16, tag="lcwt")
            nc.sync.dma_start(out=lcwt_t, in_=lcwt_d[:])
            vec_t = {}
            for nm, d in (("cab1", cab1_d), ("dwc", dwc_d), ("dwb", dwb_d),
                          ("bng", bng_d), ("bnb", bnb_d), ("bnm", bnm_d), ("bnv", bnv_d)):
                vt = consts.tile([128, 2], F32, tag=f"v_{nm}")
                nc.sync.dma_start(out=vt, in_=d[:])
                vec_t[nm] = vt
            omega_t = consts.tile([1, 129], F32, tag="omega")
            nc.sync.dma_start(out=omega_t, in_=omega_d[:])
            lam_t = consts.tile([1, 128], F32, tag="lam")
            nc.sync.dma_start(out=lam_t, in_=lam_d[:])

            # ---------------- gate MLPs (tiny), table-batched ----------------
            def mlp_head(m, xvec, nk, role):
                d = mlp_d[m]
                w1t = gsb.tile([1, 64], F32, tag="m_w1")
                nc.sync.dma_start(out=w1t, in_=d["w1t"][:])
                b1 = gsb.tile([64, 1], F32, tag="m_b1")
                nc.sync.dma_start(out=b1, in_=d["b1"][:])
                w2t = gsb.tile([64, 64], F32, tag="m_w2")
                nc.sync.dma_start(out=w2t, in_=d["w2t"][:])
                b2 = gsb.tile([64, 1], F32, tag="m_b2")
                nc.sync.dma_start(out=b2, in_=d["b2"][:])
                w3t = gsb.tile([64, 8], F32, tag="m_w3")
                nc.sync.dma_start(out=w3t, in_=d["w3t"][:])
                b3 = gsb.tile([8, 1], F32, tag="m_b3")
                nc.sync.dma_start(out=b3, in_=d["b3"][:])

                p1 = psB.tile([64, nk], F32, tag="B0")
                nc.tensor.matmul(p1, lhsT=w1t, rhs=xvec, start=True, stop=True)
                h1 = gsb.tile([64, nk], F32, tag="m_h1")
                nc.scalar.activation(h1, p1, AF.Gelu, bias=b1)
                p2 = psB.tile([64, nk], F32, tag="B1")
                nc.tensor.matmul(p2, lhsT=w2t, rhs=h1, start=True, stop=True)
                h2 = gsb.tile([64, nk], F32, tag="m_h2")
                nc.scalar.activation(h2, p2, AF.Gelu, bias=b2)
                p3 = psB.tile([8, nk], F32, tag="B0")
                nc.tensor.matmul(p3, lhsT=w3t, rhs=h2, start=True, stop=True)
                at = gsb.tile([8, nk], F32, tag=f"m_at{role}")
                nc.scalar.activation(at, p3, AF.Identity, bias=b3)
                return at

            ghh = consts.tile([128, 2, 128], F32, tag="ghh")
            ghw = consts.tile([128, 2, 128], F32, tag="ghw")
            gp = {}
            gtags = {("w", 0): "A0", ("w", 1): "A1", ("h", 0): "B0", ("h", 1): "B1"}
            for (am, bm, nmk) in (("aw", "bc2", "w"), ("ah", "bc1", "h")):
                at = mlp_head(am, omega_t, 129, "a")
                bt = mlp_head(bm, lam_t, 128, "b")
                pool = psA if nmk == "w" else psB
                g0 = pool.tile([128, 128], F32, tag=gtags[(nmk, 0)])
                nc.tensor.matmul(g0, lhsT=at[:, 0:128], rhs=bt, start=True, stop=True)
                gn = pool.tile([1, 128], F32, tag=gtags[(nmk, 1)])
                nc.tensor.matmul(gn, lhsT=at[:, 128:129], rhs=bt, start=True, stop=True)
                gp[(nmk, 0)] = g0
                gp[(nmk, 1)] = gn
            # softplus(z) = relu(z) + log1p(exp(-|z|)), stage-batched across all 4
            keys = list(gp.keys())
            sp = {}
            for i, key in enumerate(keys):
                npart = 128 if key[1] == 0 else 1
                na = gsb.tile([128, 128], F32, tag=f"sp_na{i}")
                nc.scalar.activation(na[:npart, :], gp[key], AF.Abs)
                sp[key] = na
            for i, key in enumerate(keys):
                npart = 128 if key[1] == 0 else 1
                ex = gsb.tile([128, 128], F32, tag=f"sp_ex{i}")
                nc.scalar.activation(ex[:npart, :], sp[key][:npart, :], AF.Exp, scale=-1.0)
                nc.vector.tensor_scalar_add(ex[:npart, :], ex[:npart, :], 1.0)
                sp[key] = ex
            for key in keys:
                npart = 128 if key[1] == 0 else 1
                nc.scalar.activation(sp[key][:npart, :], sp[key][:npart, :], AF.Ln)
            for i, key in enumerate(keys):
                npart = 128 if key[1] == 0 else 1
                re = gsb.tile([128, 128], F32, tag=f"sp_na{i}")
                nc.scalar.activation(re[:npart, :], gp[key], AF.Relu)
                gh = ghh if key[0] == "h" else ghw
                if key[1] == 0:
                    nc.vector.tensor_add(gh[:, 0, :], sp[key][:128, :], re[:128, :])
                else:
                    # rows 128+j of ghat equal G[j]: copy the aligned block first,
                    # then overwrite row 0 with the Nyquist G[128].
                    nc.vector.tensor_copy(gh[:, 1, :], gh[:, 0, :])
                    nc.vector.tensor_add(gh[0:1, 1, :], sp[key][0:1, :], re[0:1, :])
            for gh in (ghh, ghw):
                nc.vector.tensor_scalar_mul(gh[:, :, :], gh[:, :, :], float(8.0 ** -0.5))

            # ---------------- BN prep ----------------
            bninv = consts.tile([128, 2], F32, tag="bninv")
            nc.vector.tensor_scalar_add(bninv, vec_t["bnv"], 1e-5)
            nc.scalar.activation(bninv, bninv, AF.Sqrt)
            nc.vector.reciprocal(bninv, bninv)
            nc.vector.tensor_tensor(out=bninv, in0=vec_t["bng"], in1=bninv, op=ALU.mult)
            bnbeff = consts.tile([128, 2], F32, tag="bnbeff")
            nc.vector.tensor_tensor(out=bnbeff, in0=vec_t["bnm"], in1=bninv, op=ALU.mult)
            nc.vector.tensor_tensor(out=bnbeff, in0=vec_t["bnb"], in1=bnbeff, op=ALU.subtract)

            u_t = ubuf.tile([128, 2, 128, HQ], BF16, tag="u")    # [k, kt, c, hq|w]

            def fwd_quarter(x_dram, idx, gate):
                """Load 16c-chunked segments, forward-transform, gate into u_t."""
                for cs in range(0, 128, 16):
                    xt = []
                    for t2 in (0, 1):
                        t = xin.tile([128, 16, HQ], BF16, tag=f"x{t2}")
                        nc.sync.dma_start(out=t, in_=x_dram[idx, t2, :, cs:cs + 16, :])
                        xt.append(t)
                    for kt in (0, 1):
                        pk = psA.tile([128, 16, HQ], F32, tag=f"A{kt}", name=f"fq{kt}")
                        for ch in (0, 8):
                            for t2 in (0, 1):
                                nc.tensor.matmul(pk[:, ch:ch + 8, :],
                                                 lhsT=tfwd_t[:, t2, kt * 128:(kt + 1) * 128],
                                                 rhs=xt[t2][:, ch:ch + 8, :],
                                                 start=(t2 == 0), stop=(t2 == 1))
                        nc.vector.tensor_tensor(
                            out=u_t[:, kt, cs:cs + 16, :],
                            in0=pk,
                            in1=gate[:, kt, cs:cs + 16].unsqueeze(2)
                                .broadcast_to([128, 16, HQ]),
                            op=ALU.mult)

            # =========== WC branch in h-quarters ===========
            # pool_w partial = sum over this core's h-HALF of the DC row of u
            # (summing Tinv columns over all 256 w kills every k != 0 term);
            # the pair AllReduce then covers the full (h, w) extent.
            y_w = [ybuf.tile([128, HQ, 128], BF16, tag=f"yw{q}", name=f"yw{q}")
                   for q in range(4)]
            # DC rows land here via a transposing scatter-DMA (c -> partitions)
            # so the pool reduce runs on all 128 lanes instead of one.
            dcr_w = gsb.tile([128, 4, HQ], BF16, tag="dcrw")
            for q in range(4):
                fwd_quarter(xw_d, q, ghw)
                nc.gpsimd.dma_start(out=dcr_w[:, q, :], in_=u_t[0:1, 0, :, :])
                if q == 3:
                    pq = gsb.tile([128, 4], F32, tag="pq")
                    nc.vector.tensor_reduce(out=pq, in_=dcr_w, axis=AXX, op=ALU.add)
                    nc.vector.tensor_tensor(out=pq, in0=pq, in1=mkw_t, op=ALU.mult)
                    poolw_sb = gsb.tile([128, 1], F32, tag="poolw")
                    nc.vector.tensor_reduce(out=poolw_sb, in_=pq, axis=AXX, op=ALU.add)
                    nc.gpsimd.dma_start(out=arw_in[:], in_=poolw_sb)
                    nc.gpsimd.collective_compute(
                        "AllReduce", ALU.add,
                        replica_groups=[[0, 1], [2, 3], [4, 5], [6, 7]],
                        ins=[arw_in[:]], outs=[arw_out[:]])
                # inverse: y_w[c, hq, w]; 8 h per PSUM group, one drain each
                for hb in range(0, HQ, 8):
                    pk = psB.tile([128, 8, 128], F32, tag=f"B{(hb // 8) % 2}",
                                  name=f"wi{(hb // 8) % 2}")
                    for i in range(8):
                        for kt in (0, 1):
                            nc.tensor.matmul(pk[:, i, :], lhsT=u_t[:, kt, :, hb + i],
                                             rhs=tinvw_t[:, kt, :],
                                             start=(kt == 0), stop=(kt == 1))
                    nc.scalar.activation(y_w[q][:, hb:hb + 8, :], pk, AF.Copy)

            # =========== HC branch in w-halves ===========
            y_h = ybuf.tile([128, 256, 128], BF16, tag="yh")
            dcr_h = gsb.tile([128, 2, HQ], BF16, tag="dcrh")
            for ws in (0, 1):
                fwd_quarter(xh_d, ws, ghh)
                # DC-row trick: sum_{h,w} y_h = sqrt(N) * sum_w u[0, c, w]
                # (scatter-DMA'd before the inverse so the AR hides behind it)
                nc.gpsimd.dma_start(out=dcr_h[:, ws, :], in_=u_t[0:1, 0, :, :])
                if ws == 1:
                    ph = gsb.tile([128, 2], F32, tag="ph")
                    nc.vector.tensor_reduce(out=ph, in_=dcr_h, axis=AXX, op=ALU.add)
                    poolh_sb = gsb.tile([128, 1], F32, tag="poolh")
                    nc.vector.tensor_reduce(out=poolh_sb, in_=ph, axis=AXX, op=ALU.add)
                    nc.vector.tensor_scalar_mul(poolh_sb, poolh_sb, float(np.sqrt(N)))
                    nc.gpsimd.dma_start(out=arh_in[:], in_=poolh_sb)
                    nc.gpsimd.collective_compute(
                        "AllReduce", ALU.add,
                        replica_groups=[[0, 1], [2, 3], [4, 5], [6, 7]],
                        ins=[arh_in[:]], outs=[arh_out[:]])
                # inverse: y_h[c, h, w]; 4 w columns per PSUM group, one
                # (strided) drain each, alternating engines
                for wg in range(0, HQ, 4):
                    pk = psB.tile([128, 4, 256], F32, tag=f"B{(wg // 4) % 2}",
                                  name=f"hi{(wg // 4) % 2}")
                    for i in range(4):
                        for kt in (0, 1):
                            nc.tensor.matmul(pk[:, i, :], lhsT=u_t[:, kt, :, wg + i],
                                             rhs=tinv_t[:, kt, :],
                                             start=(kt == 0), stop=(kt == 1))
                    wl = ws * HQ + wg
                    src = pk.rearrange("c w h -> c h w")
                    if (wg // 4) % 2 == 0:
                        nc.vector.tensor_copy(y_h[:, :, wl:wl + 4], src)
                    else:
                        nc.scalar.activation(y_h[:, :, wl:wl + 4], src, AF.Copy)

            # ---------------- channel attention -> folded conv weights --------
            wsc = consts.tile([128, 2, 256], BF16, tag="wsc")
            p_sb = []
            for ct, aro in ((0, arh_out), (1, arw_out)):
                pt = gsb.tile([128, 1], F32, tag=f"p_ar{ct}")
                nc.gpsimd.dma_start(out=pt, in_=aro[:])
                p_sb.append(pt)
            q_sb = []
            for ot in (0, 1):
                q_ps = psB.tile([128, 1], F32, tag=f"B{ot}")
                for ct in (0, 1):
                    nc.tensor.matmul(q_ps, lhsT=caw1t_t[:, ct, ot * 128:(ot + 1) * 128],
                                     rhs=p_sb[ct], start=(ct == 0), stop=(ct == 1))
                qt = gsb.tile([128, 1], F32, tag=f"q{ot}")
                nc.scalar.activation(qt, q_ps, AF.Gelu, bias=vec_t["cab1"][:, ot:ot + 1])
                nc.gpsimd.tensor_tensor(out=qt, in0=qt, in1=vec_t["dwc"][:, ot:ot + 1],
                                        op=ALU.mult)
                q_sb.append(qt)
            for ot in (0, 1):
                s_t = gsb.tile([128, 1], F32, tag=f"s{ot}")
                nc.scalar.activation(s_t, q_sb[ot], AF.Sigmoid,
                                     bias=vec_t["dwb"][:, ot:ot + 1])
                nc.gpsimd.tensor_scalar_mul(wsc[:, ot, :], lcwt_t[:, ot, :], s_t)

            # ---------------- conv 1x1 + BN + GELU + residual + store ---------
            cvtags = ("A0", "A1", "B0", "B1")
            for hc in range(0, 256, 8):
                ywq = y_w[hc // HQ]
                hof = hc % HQ
                for ot in (0, 1):
                    xrt = stg.tile([128, 8, 128], BF16, tag=f"xr{ot}")
                    dq = nc.scalar if ot == 0 else nc.sync
                    dq.dma_start(out=xrt,
                                 in_=xres_d[ot * 128:(ot + 1) * 128, hc:hc + 8, :])
                    ostg = stg.tile([128, 8, 128], BF16, tag=f"g{ot}")
                    cvt = cvtags[(hc // 8 * 2 + ot) % 4]
                    po = (psA if cvt[0] == "A" else psB).tile(
                        [128, 8, 128], F32, tag=cvt, name=f"cv{cvt}")
                    for sl in (0, 4):
                        nc.tensor.matmul(po[:, sl:sl + 4, :],
                                         lhsT=wsc[:, 0, ot * 128:(ot + 1) * 128],
                                         rhs=y_h[:, hc + sl:hc + sl + 4, :],
                                         start=True, stop=False)
                        nc.tensor.matmul(po[:, sl:sl + 4, :],
                                         lhsT=wsc[:, 1, ot * 128:(ot + 1) * 128],
                                         rhs=ywq[:, hof + sl:hof + sl + 4, :],
                                         start=False, stop=True)
                    nc.scalar.activation(ostg, po, AF.Gelu,
                                         bias=bnbeff[:, ot:ot + 1],
                                         scale=bninv[:, ot:ot + 1])
                    if ot == 0 or (hc // 8) % 2 == 0:
                        nc.vector.tensor_add(ostg, ostg, xrt)
                    else:
                        nc.gpsimd.tensor_add(ostg, ostg, xrt)
                    nc.gpsimd.dma_start(out=out_d[ot * 128:(ot + 1) * 128, hc:hc + 8, :],
                                        in_=ostg)

    nc.compile()
    return nc


_NC_CACHE = None


def _get_nc():
    global _NC_CACHE
    if _NC_CACHE is None:
        _NC_CACHE = _build()
    return _NC_CACHE


def _host_consts(inputs, core):
    """Per-core constant inputs (everything except the x shards)."""
    s = core % 2
    wlo = WS * s
    T = _dft_basis()
    d = {}
    d["tfwd"] = _part_major(np.ascontiguousarray(T.T)).astype(_BF16_NP)
    d["tinv"] = _part_major(T).astype(_BF16_NP)
    d["tinvw"] = _part_major(np.ascontiguousarray(T[:, wlo:wlo + WS])).astype(_BF16_NP)
    mk = np.zeros((128, 4), np.float32)
    mk[:, 2 * s:2 * s + 2] = np.sqrt(N)
    d["mkw"] = mk
    d["omega"] = (np.arange(129, dtype=np.float32) / 128.0 - 1.0).reshape(1, 129)
    d["lam"] = np.linspace(-1.0, 1.0, 128, dtype=np.float32).reshape(1, 128)
    for m in _MLPS:
        d[f"{m}_w1t"] = np.ascontiguousarray(inputs[f"{m}_w1"].T).astype(np.float32)
        d[f"{m}_b1v"] = inputs[f"{m}_b1"].reshape(64, 1).astype(np.float32)
        d[f"{m}_w2t"] = np.ascontiguousarray(inputs[f"{m}_w2"].T).astype(np.float32)
        d[f"{m}_b2v"] = inputs[f"{m}_b2"].reshape(64, 1).astype(np.float32)
        d[f"{m}_w3t"] = np.ascontiguousarray(inputs[f"{m}_w3"].T).astype(np.float32)
        d[f"{m}_b3v"] = inputs[f"{m}_b3"].reshape(8, 1).astype(np.float32)
    d["caw1t"] = _part_major(np.ascontiguousarray(inputs["ca_w1"].T) / 65536.0).astype(np.float32)
    d["cab1"] = _part_major(inputs["ca_b1"]).astype(np.float32)
    d["dwc"] = _part_major(np.ascontiguousarray(inputs["ca_dw"][:, 1, 1])).astype(np.float32)
    d["dwb"] = _part_major(inputs["ca_db"]).astype(np.float32)
    d["lcwt"] = _part_major(np.ascontiguousarray(inputs["lc_w"].T)).astype(_BF16_NP)
    d["bng"] = _part_major(inputs["bn_g"]).astype(np.float32)
    d["bnb"] = _part_major(inputs["bn_b"]).astype(np.float32)
    d["bnm"] = _part_major(inputs["bn_m"]).astype(np.float32)
    d["bnv"] = _part_major(inputs["bn_v"]).astype(np.float32)
    return d


def kernel(**inputs):
    x = np.asarray(inputs["x"], np.float32)
    nc = _get_nc()

    in_maps = []
    for core in range(NCORES):
        b, s = core // 2, core % 2
        wlo = WS * s
        m = _host_consts(inputs, core)
        # xh: [2ws, 2ht, 128h, 128c, 64w]
        xhq = x[b, :C2, :, wlo:wlo + WS]             # (128c, 256h, 128w)
        m["xh"] = np.ascontiguousarray(
            xhq.reshape(C2, 2, 128, 2, HQ).transpose(3, 1, 2, 0, 4)).astype(_BF16_NP)
        # xw: [4q, 2wt, 128w, 128c, 64hq]
        xwq = x[b, C2:, :, :].reshape(C2, 4, HQ, N)
        m["xw"] = np.ascontiguousarray(
            xwq.transpose(1, 3, 0, 2).reshape(4, 2, 128, C2, HQ)).astype(_BF16_NP)
        m["xres"] = np.ascontiguousarray(x[b, :, :, wlo:wlo + WS]).astype(_BF16_NP)
        in_maps.append(m)

    trace = os.environ.get("BASS_KERNEL_TRACE", "0") == "1"
    res = bass_utils.run_bass_kernel_spmd(
        nc, in_maps, core_ids=list(range(NCORES)),
        trace=trace, trace_cores=list(range(NCORES)) if trace else None,
        stitch_traces=False)
    if trace and res.exec_time_ns is not None:
        print(f"HW exec time: {res.exec_time_ns} ns")
        print(f"   mean exec time: {res.mean_exec_time_ns} ns  "
              f"(slowest core {res.max_exec_time_core_id})")
        if res.instructions_and_trace is not None:
            print("   trace:", res.instructions_and_trace[1])

    out = np.empty((B, 2 * C2, N, N), np.float32)
    for core in range(NCORES):
        b, s = core // 2, core % 2
        wlo = WS * s
        out[b, :, :, wlo:wlo + WS] = res.results[core]["out"].astype(np.float32)
    return out
